# revision 19
# baseline (speedup 1.0000x reference)
"""Trainium2 Bass kernel for nn_DenseAttnProcessor (sparse_attention).

Cross-attention block: q = hs@Wq, k/v = ehs@{Wk,Wv}, per-head softmax((q k^T)/8
+ col_bias) @ v, @Wo + bo + residual.  B=8 batches -> data-parallel, one batch
per NeuronCore (no collectives).

v2 dataflow (per core):

  host prep:  hsT fp8 (pre-transposed, so no runtime DMA-transpose), hs
              residual bf16, Wq*8 fp8, Wo*16 bf16, suppression bias factored
              rank-2: ind2 [2,77] (x -SUPPRESS) and mask2 [2,HW] so that
              col_bias^T = ind2^T @ mask2 with exact set-overwrite semantics.
  stage A:    k,v = ehsT^T @ {Wk,Wv} (bf16); kT = PE-transpose(k)/64;
              M_h = v_h @ (16 Wo_h); M rows packed fp8 into [128,10,1024]
              stacked tiles (+ 16*bo row at stacked row 1232).
  stage B (8 chunks of 512 q rows, software-pipelined):
    qT   = (8Wq)^T @ hsT   -- fp8 DoubleRow, 2 k-tiles per matmul
    per head pair: scoresT [77,512] = kT_h^T qT_h (K=64) accumulated with
         the K=2 rank-2 suppression matmul (ind2 x mask2 chunk)
    z    = Exp(scoresT) on Scalar -> DMA-packed into stacked [128,10,512]
    D    = batched over all heads: 10 indicator matmuls -> [16,512] psum
    dinv = 32/D (fast DVE reciprocal, x32 folded at the bf16 copy)
    dexp = indicator-expand matmul back to [128,512] per k-tile
    prob = z * dexp  (DVE, fp8 out, = 32*softmax)
    AV   = prob^T @ M  -- fp8 DoubleRow, 5 matmuls per [128,512] psum
    out  = psum/512 + residual (fused DVE scalar_tensor_tensor), bf16 store.

AV of chunk ci is interleaved into the head loop of chunk ci+1 to keep the PE
stream dense (HAM stays warm).
"""

import sys

for _p in ("/opt/trn_rl_repo",):
    if _p not in sys.path:
        sys.path.insert(0, _p)

import numpy as np
import ml_dtypes

import concourse.mybir as mybir
import concourse.tile as tile
from concourse import bacc
from concourse.bass import ds
from concourse.masks import make_identity

F32 = mybir.dt.float32
BF16 = mybir.dt.bfloat16
FP8 = mybir.dt.float8e4
AF = mybir.ActivationFunctionType
ALU = mybir.AluOpType
DR = mybir.MatmulPerfMode.DoubleRow

B, HW, C, CT, T, H, D = 8, 4096, 1024, 2048, 77, 16, 64
SUPPRESS = 20.0
RT = H * T + 1                # 1233 stacked rows (16*77 head rows + bo row)
NKT = (RT + 127) // 128       # 10 K-tiles for the AV matmul
NQ = 512                      # q rows per chunk
NCHUNK = HW // NQ             # 8
BO_TILE, BO_PART = (H * T) // 128, (H * T) % 128   # bo/ones row: tile 9, p 80
PSC = 32.0                    # probs scale (fp8 range)
MSC = 16.0                    # M scale (fp8 range)
QSC = 64.0                    # q scale (Wq*scale*64 fp8, kT/64)


def _pack_pieces(h):
    """DMA pieces for packing head h's 77 rows at stacked row 77*h, split at
    128-row tile boundaries.  Returns list of (tile_idx, part_base, src_start,
    nrows)."""
    g = T * h
    pieces = []
    pos = 0
    while pos < T:
        gg = g + pos
        ti, d = gg // 128, gg % 128
        n = min(T - pos, 128 - d)
        pieces.append((ti, d, pos, n))
        pos += n
    return pieces


def build_nc():
    nc = bacc.Bacc("TRN2", target_bir_lowering=False, debug=False)

    hsT = nc.dram_tensor("hsT", [C, HW], FP8, kind="ExternalInput")
    hsres = nc.dram_tensor("hsres", [HW, C], BF16, kind="ExternalInput")
    ehsT = nc.dram_tensor("ehsT", [CT, T], FP8, kind="ExternalInput")
    wq = nc.dram_tensor("wq", [C, C], FP8, kind="ExternalInput")
    wk = nc.dram_tensor("wk", [CT, C], FP8, kind="ExternalInput")
    wv = nc.dram_tensor("wv", [CT, C], FP8, kind="ExternalInput")
    wo = nc.dram_tensor("wo", [C, C], BF16, kind="ExternalInput")
    bo = nc.dram_tensor("bo", [1, C], FP8, kind="ExternalInput")
    ind2 = nc.dram_tensor("ind2", [2, T], BF16, kind="ExternalInput")
    mask2 = nc.dram_tensor("mask2", [2, HW], BF16, kind="ExternalInput")
    etd = nc.dram_tensor("etd", [128, NKT * 32], FP8, kind="ExternalInput")
    exp_ind = nc.dram_tensor("exp_ind", [17, NKT * 128], BF16, kind="ExternalInput")
    out = nc.dram_tensor("out", [HW, C], BF16, kind="ExternalOutput")

    with tile.TileContext(nc) as tc:
        with (
            tc.tile_pool(name="const", bufs=1) as const,
            tc.tile_pool(name="persist", bufs=1) as persist,
        ):
            ident = const.tile([128, 128], BF16)
            make_identity(nc, ident)
            ind2_sb = const.tile([2, T], BF16)
            nc.sync.dma_start(ind2_sb, ind2[:, :])
            mask2_sb = const.tile([2, HW], BF16)
            nc.sync.dma_start(mask2_sb, mask2[:, :])
            etd_sb = const.tile([128, NKT, 32], FP8)
            nc.sync.dma_start(etd_sb, etd[:, :])
            ex_sb = const.tile([49, NKT, 128], BF16)
            nc.sync.dma_start(ex_sb[ds(0, 17), :, :], exp_ind[:, :])
            nc.sync.dma_start(ex_sb[ds(32, 17), :, :], exp_ind[:, :])

            # persistent stacks
            kT_sb = persist.tile([128, C // 128, T], BF16)        # [inner, t]
            m_f8 = persist.tile([128, NKT, C], FP8)               # stacked 16*M
            wq_sb = persist.tile([128, C // 128, C], FP8)
            for i in range(C // 128):
                nc.sync.dma_start(wq_sb[:, i, :], wq[ds(128 * i, 128), :])
            z_bufs = [persist.tile([128, NKT, NQ], FP8, name=f"z{b}") for b in range(2)]
            prob_bufs = [persist.tile([128, NKT, NQ], FP8, name=f"pb{b}") for b in range(2)]
            psc_row = const.tile([1, NQ], FP8)
            nc.any.memset(psc_row, PSC)
            for zb in z_bufs:
                # bo/ones pseudo-row = PSC; rows past it zero (NaN hygiene for
                # the D matmul which reads all 128 partitions).  memset can
                # only start at 32-aligned partitions; DMA patches row 80.
                nc.any.memset(zb[ds(64, 64), BO_TILE, :], 0.0)
                nc.sync.dma_start(zb[ds(BO_PART, 1), BO_TILE, :], psc_row)
            # M stack tile 9: rows past head rows; bo*16 at BO_PART
            nc.any.memset(m_f8[ds(64, 64), BO_TILE, :], 0.0)
            nc.sync.dma_start(m_f8[ds(BO_PART, 1), BO_TILE, :], bo[:, :])

            st = {}

            with (
                tc.tile_pool(name="hsp", bufs=2) as hsp,
                tc.tile_pool(name="work", bufs=2) as work,
                tc.tile_pool(name="soft", bufs=4) as soft,
            ):

                def load(ci):
                    q0 = NQ * ci
                    hsT_t = hsp.tile([128, C // 128, NQ], FP8, tag="hsT")
                    for cj in range(C // 128):
                        nc.sync.dma_start(
                            hsT_t[:, cj, :], hsT[ds(128 * cj, 128), ds(q0, NQ)]
                        )
                    res_t = hsp.tile([128, NQ // 128, C], BF16, tag="res", bufs=3)
                    for qj in range(NQ // 128):
                        nc.sync.dma_start(
                            res_t[:, qj, :], hsres[ds(q0 + 128 * qj, 128), :]
                        )
                    qT = work.tile([128, C // 128, NQ], BF16, tag="qT")
                    st[ci] = dict(hsT=hsT_t, res=res_t, qT=qT)

                def qt_group(ci, ij, ps_pool, tag="qps", bufs=1):
                    hsT_t, qT = st[ci]["hsT"], st[ci]["qT"]
                    q_ps = ps_pool.tile([128, NQ], F32, tag=tag, bufs=bufs)
                    for c2 in range(C // 256):
                        nc.tensor.matmul(
                            q_ps,
                            wq_sb[:, ds(2 * c2, 2), ds(128 * ij, 128)],
                            hsT_t[:, ds(2 * c2, 2), :],
                            start=(c2 == 0),
                            stop=(c2 == C // 256 - 1),
                            perf_mode=DR,
                        )
                    nc.any.tensor_copy(qT[:, ij, :], q_ps)

                # ---------------- stage A: k, v, kT, M ----------------
                with (
                    tc.tile_pool(name="sa_sb", bufs=3) as sa_sb,
                    tc.tile_pool(name="sa_w", bufs=4) as sa_w,
                    tc.tile_pool(name="sa_ps", bufs=2, space="PSUM") as sa_ps,
                ):
                    # chunk-0 loads + qT(0) early so PE/DMA warm up while the
                    # k/v weight tiles stream in.  "big" [128,1024] psum tag is
                    # shared by qT(0) (first 512 cols) and the M matmuls.
                    load(0)
                    ehsT_sb = sa_sb.tile([128, CT // 128, 80], FP8, bufs=1)
                    nc.any.memset(ehsT_sb, 0.0)
                    for j in range(CT // 128):
                        nc.sync.dma_start(
                            ehsT_sb[:, j, ds(0, T)], ehsT[ds(128 * j, 128), :]
                        )

                    def big_ps():
                        return sa_ps.tile([128, C], F32, tag="big", bufs=2, name="bigps")

                    for ij in range(C // 128):
                        hsT_t, qT = st[0]["hsT"], st[0]["qT"]
                        q_ps = big_ps()
                        for c2 in range(C // 256):
                            nc.tensor.matmul(
                                q_ps[:, ds(0, NQ)],
                                wq_sb[:, ds(2 * c2, 2), ds(128 * ij, 128)],
                                hsT_t[:, ds(2 * c2, 2), :],
                                start=(c2 == 0),
                                stop=(c2 == C // 256 - 1),
                                perf_mode=DR,
                            )
                        nc.any.tensor_copy(qT[:, ij, :], q_ps[:, ds(0, NQ)])

                    kv_sb = {}
                    for name, wten in (("k", wk), ("v", wv)):
                        kv_ps = sa_ps.tile([80, C], F32, tag="kvps", bufs=1)
                        wt = sa_w.tile([128, CT // 128, C], FP8, tag=f"w{name}", bufs=1)
                        for j in range(CT // 128):
                            for qtr in range(4):
                                nc.sync.dma_start(
                                    wt[:, j, ds(256 * qtr, 256)],
                                    wten[ds(128 * j, 128), ds(256 * qtr, 256)],
                                )
                        for nh in range(2):
                            for j2 in range(CT // 256):
                                nc.tensor.matmul(
                                    kv_ps[:, ds(512 * nh, 512)],
                                    ehsT_sb[:, ds(2 * j2, 2), :],
                                    wt[:, ds(2 * j2, 2), ds(512 * nh, 512)],
                                    start=(j2 == 0),
                                    stop=(j2 == CT // 256 - 1),
                                    perf_mode=DR,
                                )
                        kvs = sa_sb.tile([T, C], BF16, tag=f"{name}sb", bufs=1)
                        # host scales Wk/Wv by 64 for fp8 range; fold out here
                        # (k also folds 1/QSC so scoresT = (k/64)^T (64 q/8))
                        sc = 1.0 / (QSC * 64.0) if name == "k" else 1.0 / 64.0
                        nc.scalar.activation(kvs, kv_ps[ds(0, T), :], AF.Copy, scale=sc)
                        kv_sb[name] = kvs

                    # kT / vT via PE transpose of 128-column slices
                    vT_sb = sa_sb.tile([128, C // 128, T], BF16, bufs=1)
                    for src, dst in ((kv_sb["k"], kT_sb), (kv_sb["v"], vT_sb)):
                        for i in range(C // 128):
                            tp = sa_ps.tile([128, T], BF16, tag="tpa")
                            nc.tensor.transpose(tp, src[:, ds(128 * i, 128)], ident[:T, :T])
                            nc.any.tensor_copy(dst[:, i, :], tp)

                    # M_h = v_h @ (16 Wo_h), fp8-packed at stacked row 77h.
                    # Head pairs run in disjoint row-strips on two psum banks.
                    for i in range(C // 128):
                        wot = sa_w.tile([128, C], BF16, tag="wot")
                        nc.sync.dma_start(wot[:, ds(0, 512)], wo[ds(128 * i, 128), ds(0, 512)])
                        nc.sync.dma_start(
                            wot[:, ds(512, 512)], wo[ds(128 * i, 128), ds(512, 512)]
                        )
                        mps = [big_ps(), big_ps()]
                        for nh in range(2):
                            for sub in range(2):
                                nc.tensor.matmul(
                                    mps[sub][ds(0, T), ds(512 * nh, 512)],
                                    vT_sb[ds(64 * sub, 64), i, :],
                                    wot[ds(64 * sub, 64), ds(512 * nh, 512)],
                                    start=True,
                                    stop=True,
                                )
                        for sub in range(2):
                            h = 2 * i + sub
                            m_stg = sa_sb.tile([T, C], FP8, tag="mstg")
                            nc.any.tensor_copy(m_stg, mps[sub][ds(0, T), :])
                            for (ti, pb, s0, nr) in _pack_pieces(h):
                                nc.gpsimd.dma_start(
                                    m_f8[ds(pb, nr), ti, :], m_stg[ds(s0, nr), :]
                                )

                # ---------------- stage B ----------------
                with tc.tile_pool(name="ops", bufs=2, space="PSUM") as ops:

                    def chunk_eu(ci):
                        # shared suppression factor exp(col_bias)^T for this
                        # chunk: one K=2 matmul + one Exp, used by all heads
                        q0 = NQ * ci
                        b_ps = ops.tile([T, NQ], F32, tag="sT", bufs=2, name="bps")
                        nc.tensor.matmul(
                            b_ps, ind2_sb, mask2_sb[:, ds(q0, NQ)], start=True, stop=True
                        )
                        euT = soft.tile([T, NQ], BF16, tag="euT", bufs=2)
                        nc.scalar.activation(euT, b_ps, AF.Exp)
                        return euT

                    def sm_pair(ci, pair, d_state, euT):
                        qT = st[ci]["qT"]
                        zb = z_bufs[ci % 2]
                        sps = []
                        for sub in range(2):
                            po = 64 * sub
                            sT_ps = ops.tile([T, NQ], F32, tag="sT", bufs=2)
                            nc.tensor.matmul(
                                sT_ps,
                                kT_sb[ds(po, 64), pair, :],
                                qT[ds(po, 64), pair, :],
                                start=True,
                                stop=True,
                            )
                            sps.append(sT_ps)
                        for sub in range(2):
                            h = 2 * pair + sub
                            z_h = soft.tile([T, NQ], FP8, tag="zh", bufs=4)
                            nc.scalar.activation(z_h, sps[sub], AF.Exp)
                            z2 = soft.tile([T, NQ], FP8, tag="z2", bufs=4)
                            nc.vector.tensor_mul(z2, z_h, euT)
                            for (ti, pb, s0, nr) in _pack_pieces(h):
                                nc.gpsimd.dma_start(
                                    zb[ds(pb, nr), ti, :], z2[ds(s0, nr), :]
                                )
                        # D matmuls (DoubleRow over k-tile pairs) as soon as
                        # both tiles of a pair are fully packed
                        zrows = 154 * (pair + 1)
                        while (
                            d_state["kt"] < NKT
                            and (128 * (d_state["kt"] + 2) <= zrows or pair == 7)
                        ):
                            kt = d_state["kt"]
                            nc.tensor.matmul(
                                d_state["ps"],
                                etd_sb[:, ds(kt, 2), :],
                                zb[:, ds(kt, 2), :],
                                start=(kt == 0),
                                stop=(kt == NKT - 2),
                                perf_mode=DR,
                            )
                            d_state["kt"] += 2

                    def emit_dinv(d_state):
                        dinv = soft.tile([16, NQ], F32, tag="dinv", bufs=2)
                        nc.vector.reciprocal_approx_fast(dinv, d_state["ps"][ds(0, 16), :])
                        dinv_bf = soft.tile([49, NQ], BF16, tag="dinvbf", bufs=2)
                        nc.any.memset(dinv_bf, 1.0)
                        nc.scalar.activation(
                            dinv_bf[ds(0, 16), :], dinv, AF.Copy, scale=PSC
                        )
                        nc.scalar.activation(
                            dinv_bf[ds(32, 16), :], dinv, AF.Copy, scale=PSC
                        )
                        return dinv_bf

                    def expand_norm(ci, dinv_bf):
                        zb = z_bufs[ci % 2]
                        pb = prob_bufs[ci % 2]
                        for kt in range(NKT):
                            # alternate row-strips 0/1 so consecutive expand
                            # matmuls overlap in the PE array
                            po = 32 * (kt % 2)
                            dexp_ps = ops.tile([128, NQ], F32, tag="dexp", bufs=2)
                            nc.tensor.matmul(
                                dexp_ps,
                                ex_sb[ds(po, 17), kt, :],
                                dinv_bf[ds(po, 17), :],
                                start=True,
                                stop=True,
                            )
                            nc.vector.tensor_mul(pb[:, kt, :], zb[:, kt, :], dexp_ps)

                    def av_group(ci, g):
                        q0 = NQ * ci
                        qj, nh = g // 2, g % 2
                        pb = prob_bufs[ci % 2]
                        res_t = st[ci]["res"]
                        o_ps = ops.tile([128, 512], F32, tag="ops", bufs=2)
                        for p5 in range(NKT // 2):
                            nc.tensor.matmul(
                                o_ps,
                                pb[:, ds(2 * p5, 2), ds(128 * qj, 128)],
                                m_f8[:, ds(2 * p5, 2), ds(512 * nh, 512)],
                                start=(p5 == 0),
                                stop=(p5 == NKT // 2 - 1),
                                perf_mode=DR,
                            )
                        if nh == 0:
                            st[ci][f"osb{qj}"] = work.tile(
                                [128, C], BF16, tag="osb", bufs=3, name=f"osb{ci}_{qj}"
                            )
                        o_sb = st[ci][f"osb{qj}"]
                        nc.vector.scalar_tensor_tensor(
                            o_sb[:, ds(512 * nh, 512)],
                            o_ps,
                            1.0 / (PSC * MSC),
                            res_t[:, qj, ds(512 * nh, 512)],
                            op0=ALU.mult,
                            op1=ALU.add,
                        )
                        if nh == 1:
                            nc.sync.dma_start(
                                out[ds(q0 + 128 * qj, 128), :], o_sb
                            )

                    for ci in range(NCHUNK):
                        if ci + 1 < NCHUNK:
                            load(ci + 1)
                        d_state = {
                            "kt": 0,
                            "ps": ops.tile([32, NQ], F32, tag="dps", bufs=1, name="dps"),
                        }
                        euT = chunk_eu(ci)
                        for pair in range(H // 2):
                            if ci > 0:
                                av_group(ci - 1, pair)
                            sm_pair(ci, pair, d_state, euT)
                            if pair < 6 and ci + 1 < NCHUNK:
                                qt_group(ci + 1, pair, ops)
                        dinv_bf = emit_dinv(d_state)
                        if ci + 1 < NCHUNK:
                            qt_group(ci + 1, 6, ops)
                            qt_group(ci + 1, 7, ops)
                        expand_norm(ci, dinv_bf)
                    for g in range(8):
                        av_group(NCHUNK - 1, g)

    nc.compile()
    return nc


_NC_CACHE = {}


def get_nc():
    if "nc" not in _NC_CACHE:
        _NC_CACHE["nc"] = build_nc()
    return _NC_CACHE["nc"]


def _bf16(x):
    return np.asarray(x, dtype=ml_dtypes.bfloat16)


def _fp8(x):
    return np.clip(np.asarray(x, np.float32), -240.0, 240.0).astype(
        ml_dtypes.float8_e4m3
    )


def make_in_maps(inputs):
    hs = np.asarray(inputs["hidden_states"], dtype=np.float32)
    ehs = np.asarray(inputs["encoder_hidden_states"], dtype=np.float32)
    mask_A = np.asarray(inputs["mask_A"], dtype=np.float32)
    mask_B = np.asarray(inputs["mask_B"], dtype=np.float32)
    Wq = np.asarray(inputs["Wq"], dtype=np.float32)
    Wk = np.asarray(inputs["Wk"], dtype=np.float32)
    Wv = np.asarray(inputs["Wv"], dtype=np.float32)
    Wo = np.asarray(inputs["Wo"], dtype=np.float32)
    bo = np.asarray(inputs["bo"], dtype=np.float32)
    idxA = np.asarray(inputs["token_indices_A"]).astype(np.int64) % T
    idxB = np.asarray(inputs["token_indices_B"]).astype(np.int64) % T

    # rank-2 suppression: bias[t,q] = ind2[:,t] . mask2[:,q], with B-set
    # overwriting A-set (reference applies A then B)
    inA = np.zeros(T, np.float32)
    inA[idxA] = 1.0
    inB = np.zeros(T, np.float32)
    inB[idxB] = 1.0
    ind2_np = np.stack([-SUPPRESS * inA * (1.0 - inB), -SUPPRESS * inB])
    mask2_np = np.stack([1.0 - mask_A, 1.0 - mask_B])

    # D-sum indicator [p, kt*16+h] and expand indicator [h(17), kt*128+p]
    rows = np.arange(NKT * 128)
    head_of = np.where(rows < H * T, rows // T, -1)
    etd_np = np.zeros((128, NKT * 32), np.float32)
    ex_np = np.zeros((17, NKT * 128), np.float32)
    for kt in range(NKT):
        for p in range(128):
            hh = head_of[kt * 128 + p]
            if 0 <= hh < H:
                etd_np[p, kt * 32 + hh] = 1.0
                ex_np[hh, kt * 128 + p] = 1.0
    ex_np[16, BO_TILE * 128 + BO_PART] = 1.0

    wq_f8 = _fp8(Wq * (QSC / np.sqrt(D)))
    wk_f8, wv_f8 = _fp8(Wk * 64.0), _fp8(Wv * 64.0)
    wo_bf = _bf16(Wo * MSC)
    bo_f8 = _fp8(bo * MSC)[None, :]
    ind2_bf, mask2_bf = _bf16(ind2_np), _bf16(mask2_np)
    etd_bf, ex_bf = _fp8(etd_np), _bf16(ex_np)

    in_maps = []
    for b in range(B):
        in_maps.append(
            {
                "hsT": _fp8(hs[b].T),
                "hsres": _bf16(hs[b]),
                "ehsT": _fp8(ehs[b].T.copy()),
                "wq": wq_f8,
                "wk": wk_f8,
                "wv": wv_f8,
                "wo": wo_bf,
                "bo": bo_f8,
                "ind2": ind2_bf,
                "mask2": mask2_bf,
                "etd": etd_bf,
                "exp_ind": ex_bf,
            }
        )
    return in_maps


def kernel(**inputs) -> np.ndarray:
    from concourse.bass_utils import run_bass_kernel_spmd

    nc = get_nc()
    in_maps = make_in_maps(inputs)
    res = run_bass_kernel_spmd(nc, in_maps, core_ids=list(range(B)))
    return np.stack([res.results[b]["out"] for b in range(B)]).astype(np.float32)


# revision 20
# speedup vs baseline: 1.0948x; 1.0948x over previous
"""Trainium2 Bass kernel for nn_DenseAttnProcessor (sparse_attention).

Cross-attention block: q = hs@Wq, k/v = ehs@{Wk,Wv}, per-head softmax((q k^T)/8
+ col_bias) @ v, @Wo + bo + residual.  B=8 batches -> data-parallel, one batch
per NeuronCore (no collectives).

v2 dataflow (per core):

  host prep:  hsT fp8 (pre-transposed, so no runtime DMA-transpose), hs
              residual bf16, Wq*8 fp8, Wo*16 bf16, suppression bias factored
              rank-2: ind2 [2,77] (x -SUPPRESS) and mask2 [2,HW] so that
              col_bias^T = ind2^T @ mask2 with exact set-overwrite semantics.
  stage A:    k,v = ehsT^T @ {Wk,Wv} (bf16); kT = PE-transpose(k)/64;
              M_h = v_h @ (16 Wo_h); M rows packed fp8 into [128,10,1024]
              stacked tiles (+ 16*bo row at stacked row 1232).
  stage B (8 chunks of 512 q rows, software-pipelined):
    qT   = (8Wq)^T @ hsT   -- fp8 DoubleRow, 2 k-tiles per matmul
    per head pair: scoresT [77,512] = kT_h^T qT_h (K=64) accumulated with
         the K=2 rank-2 suppression matmul (ind2 x mask2 chunk)
    z    = Exp(scoresT) on Scalar -> DMA-packed into stacked [128,10,512]
    D    = batched over all heads: 10 indicator matmuls -> [16,512] psum
    dinv = 32/D (fast DVE reciprocal, x32 folded at the bf16 copy)
    dexp = indicator-expand matmul back to [128,512] per k-tile
    prob = z * dexp  (DVE, fp8 out, = 32*softmax)
    AV   = prob^T @ M  -- fp8 DoubleRow, 5 matmuls per [128,512] psum
    out  = psum/512 + residual (fused DVE scalar_tensor_tensor), bf16 store.

AV of chunk ci is interleaved into the head loop of chunk ci+1 to keep the PE
stream dense (HAM stays warm).
"""

import sys

for _p in ("/opt/trn_rl_repo",):
    if _p not in sys.path:
        sys.path.insert(0, _p)

import numpy as np
import ml_dtypes

import concourse.mybir as mybir
import concourse.tile as tile
from concourse import bacc
from concourse.bass import ds
from concourse.masks import make_identity

F32 = mybir.dt.float32
BF16 = mybir.dt.bfloat16
FP8 = mybir.dt.float8e4
AF = mybir.ActivationFunctionType
ALU = mybir.AluOpType
DR = mybir.MatmulPerfMode.DoubleRow

B, HW, C, CT, T, H, D = 8, 4096, 1024, 2048, 77, 16, 64
SUPPRESS = 20.0
RT = H * T + 1                # 1233 stacked rows (16*77 head rows + bo row)
NKT = (RT + 127) // 128       # 10 K-tiles for the AV matmul
NQ = 512                      # q rows per chunk
NCHUNK = HW // NQ             # 8
BO_TILE, BO_PART = (H * T) // 128, (H * T) % 128   # bo/ones row: tile 9, p 80
PSC = 32.0                    # probs scale (fp8 range)
MSC = 16.0                    # M scale (fp8 range)
QSC = 64.0                    # q scale (Wq*scale*64 fp8, kT/64)


def _pack_pieces(h):
    """DMA pieces for packing head h's 77 rows at stacked row 77*h, split at
    128-row tile boundaries.  Returns list of (tile_idx, part_base, src_start,
    nrows)."""
    g = T * h
    pieces = []
    pos = 0
    while pos < T:
        gg = g + pos
        ti, d = gg // 128, gg % 128
        n = min(T - pos, 128 - d)
        pieces.append((ti, d, pos, n))
        pos += n
    return pieces


def build_nc():
    nc = bacc.Bacc("TRN2", target_bir_lowering=False, debug=False)

    hsT = nc.dram_tensor("hsT", [C, HW], FP8, kind="ExternalInput")
    hsres = nc.dram_tensor("hsres", [HW, C], BF16, kind="ExternalInput")
    ehsT = nc.dram_tensor("ehsT", [CT, T], FP8, kind="ExternalInput")
    wq = nc.dram_tensor("wq", [C, C], FP8, kind="ExternalInput")
    wk = nc.dram_tensor("wk", [CT, C], FP8, kind="ExternalInput")
    wv = nc.dram_tensor("wv", [CT, C], FP8, kind="ExternalInput")
    wo = nc.dram_tensor("wo", [C, C], BF16, kind="ExternalInput")
    bo = nc.dram_tensor("bo", [1, C], FP8, kind="ExternalInput")
    ind2 = nc.dram_tensor("ind2", [2, T], BF16, kind="ExternalInput")
    mask2 = nc.dram_tensor("mask2", [2, HW], BF16, kind="ExternalInput")
    etd = nc.dram_tensor("etd", [128, NKT * 32], FP8, kind="ExternalInput")
    exp_ind = nc.dram_tensor("exp_ind", [17, NKT * 128], BF16, kind="ExternalInput")
    out = nc.dram_tensor("out", [HW, C], BF16, kind="ExternalOutput")

    with tile.TileContext(nc) as tc:
        with (
            tc.tile_pool(name="const", bufs=1) as const,
            tc.tile_pool(name="persist", bufs=1) as persist,
        ):
            ident = const.tile([128, 128], BF16)
            make_identity(nc, ident)
            ind2_sb = const.tile([2, T], BF16)
            nc.sync.dma_start(ind2_sb, ind2[:, :])
            mask2_sb = const.tile([2, HW], BF16)
            nc.sync.dma_start(mask2_sb, mask2[:, :])
            etd_sb = const.tile([128, NKT, 32], FP8)
            nc.sync.dma_start(etd_sb, etd[:, :])
            ex_sb = const.tile([49, NKT, 128], BF16)
            nc.sync.dma_start(ex_sb[ds(0, 17), :, :], exp_ind[:, :])
            nc.sync.dma_start(ex_sb[ds(32, 17), :, :], exp_ind[:, :])

            # persistent stacks
            kT_sb = persist.tile([128, C // 128, T], BF16)        # [inner, t]
            m_f8 = persist.tile([128, NKT, C], FP8)               # stacked 16*M
            wq_sb = persist.tile([128, C // 128, C], FP8)
            for i in range(C // 128):
                nc.sync.dma_start(wq_sb[:, i, :], wq[ds(128 * i, 128), :])
            z_bufs = [persist.tile([128, NKT, NQ], FP8, name=f"z{b}") for b in range(2)]
            prob_bufs = [persist.tile([128, NKT, NQ], FP8, name=f"pb{b}") for b in range(2)]
            psc_row = const.tile([1, NQ], FP8)
            nc.any.memset(psc_row, PSC)
            for zb in z_bufs:
                # bo/ones pseudo-row = PSC; rows past it zero (NaN hygiene for
                # the D matmul which reads all 128 partitions).  memset can
                # only start at 32-aligned partitions; DMA patches row 80.
                nc.any.memset(zb[ds(64, 64), BO_TILE, :], 0.0)
                nc.sync.dma_start(zb[ds(BO_PART, 1), BO_TILE, :], psc_row)
            # M stack tile 9: rows past head rows; bo*16 at BO_PART
            nc.any.memset(m_f8[ds(64, 64), BO_TILE, :], 0.0)
            nc.sync.dma_start(m_f8[ds(BO_PART, 1), BO_TILE, :], bo[:, :])

            st = {}

            with (
                tc.tile_pool(name="hsp", bufs=2) as hsp,
                tc.tile_pool(name="work", bufs=2) as work,
                tc.tile_pool(name="soft", bufs=4) as soft,
            ):

                def load(ci):
                    q0 = NQ * ci
                    hsT_t = hsp.tile([128, C // 128, NQ], FP8, tag="hsT")
                    for cj in range(C // 128):
                        nc.sync.dma_start(
                            hsT_t[:, cj, :], hsT[ds(128 * cj, 128), ds(q0, NQ)]
                        )
                    res_t = hsp.tile([128, NQ // 128, C], BF16, tag="res", bufs=3)
                    for qj in range(NQ // 128):
                        nc.sync.dma_start(
                            res_t[:, qj, :], hsres[ds(q0 + 128 * qj, 128), :]
                        )
                    qT = work.tile([128, C // 128, NQ], BF16, tag="qT")
                    st[ci] = dict(hsT=hsT_t, res=res_t, qT=qT)

                def qt_group(ci, ij, ps_pool, tag="qps", bufs=1):
                    hsT_t, qT = st[ci]["hsT"], st[ci]["qT"]
                    q_ps = ps_pool.tile([128, NQ], F32, tag=tag, bufs=bufs)
                    for c2 in range(C // 256):
                        nc.tensor.matmul(
                            q_ps,
                            wq_sb[:, ds(2 * c2, 2), ds(128 * ij, 128)],
                            hsT_t[:, ds(2 * c2, 2), :],
                            start=(c2 == 0),
                            stop=(c2 == C // 256 - 1),
                            perf_mode=DR,
                        )
                    nc.any.tensor_copy(qT[:, ij, :], q_ps)

                # ---------------- stage A: k, v, kT, M ----------------
                with (
                    tc.tile_pool(name="sa_sb", bufs=3) as sa_sb,
                    tc.tile_pool(name="sa_w", bufs=4) as sa_w,
                    tc.tile_pool(name="sa_ps", bufs=2, space="PSUM") as sa_ps,
                ):
                    # chunk-0 loads + qT(0) early so PE/DMA warm up while the
                    # k/v weight tiles stream in.  "big" [128,1024] psum tag is
                    # shared by qT(0) (first 512 cols) and the M matmuls.
                    load(0)
                    ehsT_sb = sa_sb.tile([128, CT // 128, 80], FP8, bufs=1)
                    nc.any.memset(ehsT_sb, 0.0)
                    for j in range(CT // 128):
                        nc.sync.dma_start(
                            ehsT_sb[:, j, ds(0, T)], ehsT[ds(128 * j, 128), :]
                        )

                    def big_ps():
                        return sa_ps.tile([128, C], F32, tag="big", bufs=2, name="bigps")

                    for ij in range(C // 128):
                        hsT_t, qT = st[0]["hsT"], st[0]["qT"]
                        q_ps = big_ps()
                        for c2 in range(C // 256):
                            nc.tensor.matmul(
                                q_ps[:, ds(0, NQ)],
                                wq_sb[:, ds(2 * c2, 2), ds(128 * ij, 128)],
                                hsT_t[:, ds(2 * c2, 2), :],
                                start=(c2 == 0),
                                stop=(c2 == C // 256 - 1),
                                perf_mode=DR,
                            )
                        nc.any.tensor_copy(qT[:, ij, :], q_ps[:, ds(0, NQ)])

                    kv_sb = {}
                    for name, wten in (("k", wk), ("v", wv)):
                        kv_ps = sa_ps.tile([80, C], F32, tag="kvps", bufs=1)
                        wt = sa_w.tile([128, CT // 128, C], FP8, tag=f"w{name}", bufs=1)
                        for j in range(CT // 128):
                            nc.sync.dma_start(
                                wt[:, j, ds(0, 512)], wten[ds(128 * j, 128), ds(0, 512)]
                            )
                            nc.sync.dma_start(
                                wt[:, j, ds(512, 512)],
                                wten[ds(128 * j, 128), ds(512, 512)],
                            )
                        for nh in range(2):
                            for j2 in range(CT // 256):
                                nc.tensor.matmul(
                                    kv_ps[:, ds(512 * nh, 512)],
                                    ehsT_sb[:, ds(2 * j2, 2), :],
                                    wt[:, ds(2 * j2, 2), ds(512 * nh, 512)],
                                    start=(j2 == 0),
                                    stop=(j2 == CT // 256 - 1),
                                    perf_mode=DR,
                                )
                        kvs = sa_sb.tile([T, C], BF16, tag=f"{name}sb", bufs=1)
                        # host scales Wk/Wv by 64 for fp8 range; fold out here
                        # (k also folds 1/QSC so scoresT = (k/64)^T (64 q/8))
                        sc = 1.0 / (QSC * 64.0) if name == "k" else 1.0 / 64.0
                        nc.scalar.activation(kvs, kv_ps[ds(0, T), :], AF.Copy, scale=sc)
                        kv_sb[name] = kvs

                    # kT / vT via PE transpose of 128-column slices
                    vT_sb = sa_sb.tile([128, C // 128, T], BF16, bufs=1)
                    for src, dst in ((kv_sb["k"], kT_sb), (kv_sb["v"], vT_sb)):
                        for i in range(C // 128):
                            tp = sa_ps.tile([128, T], BF16, tag="tpa")
                            nc.tensor.transpose(tp, src[:, ds(128 * i, 128)], ident[:T, :T])
                            nc.any.tensor_copy(dst[:, i, :], tp)

                    # M_h = v_h @ (16 Wo_h), fp8-packed at stacked row 77h.
                    # Head pairs run in disjoint row-strips on two psum banks.
                    for i in range(C // 128):
                        wot = sa_w.tile([128, C], BF16, tag="wot")
                        nc.sync.dma_start(wot[:, ds(0, 512)], wo[ds(128 * i, 128), ds(0, 512)])
                        nc.sync.dma_start(
                            wot[:, ds(512, 512)], wo[ds(128 * i, 128), ds(512, 512)]
                        )
                        mps = [big_ps(), big_ps()]
                        for nh in range(2):
                            for sub in range(2):
                                nc.tensor.matmul(
                                    mps[sub][ds(0, T), ds(512 * nh, 512)],
                                    vT_sb[ds(64 * sub, 64), i, :],
                                    wot[ds(64 * sub, 64), ds(512 * nh, 512)],
                                    start=True,
                                    stop=True,
                                )
                        for sub in range(2):
                            h = 2 * i + sub
                            m_stg = sa_sb.tile([T, C], FP8, tag="mstg")
                            nc.any.tensor_copy(m_stg, mps[sub][ds(0, T), :])
                            for (ti, pb, s0, nr) in _pack_pieces(h):
                                nc.gpsimd.dma_start(
                                    m_f8[ds(pb, nr), ti, :], m_stg[ds(s0, nr), :]
                                )

                # ---------------- stage B ----------------
                with tc.tile_pool(name="ops", bufs=2, space="PSUM") as ops:

                    def chunk_eu(ci):
                        # shared suppression factor exp(col_bias)^T for this
                        # chunk: one K=2 matmul + one Exp, used by all heads
                        q0 = NQ * ci
                        b_ps = ops.tile([T, NQ], F32, tag="sT", bufs=2, name="bps")
                        nc.tensor.matmul(
                            b_ps, ind2_sb, mask2_sb[:, ds(q0, NQ)], start=True, stop=True
                        )
                        euT = soft.tile([T, NQ], BF16, tag="euT", bufs=2)
                        nc.scalar.activation(euT, b_ps, AF.Exp)
                        return euT

                    def sm_pair(ci, pair, d_state, euT):
                        qT = st[ci]["qT"]
                        zb = z_bufs[ci % 2]
                        sps = []
                        for sub in range(2):
                            po = 64 * sub
                            sT_ps = ops.tile([T, NQ], F32, tag="sT", bufs=2)
                            nc.tensor.matmul(
                                sT_ps,
                                kT_sb[ds(po, 64), pair, :],
                                qT[ds(po, 64), pair, :],
                                start=True,
                                stop=True,
                            )
                            sps.append(sT_ps)
                        for sub in range(2):
                            h = 2 * pair + sub
                            z_h = soft.tile([T, NQ], FP8, tag="zh", bufs=4)
                            nc.scalar.activation(z_h, sps[sub], AF.Exp)
                            z2 = soft.tile([T, NQ], FP8, tag="z2", bufs=4)
                            nc.vector.tensor_mul(z2, z_h, euT)
                            for (ti, pb, s0, nr) in _pack_pieces(h):
                                nc.gpsimd.dma_start(
                                    zb[ds(pb, nr), ti, :], z2[ds(s0, nr), :]
                                )
                        # D matmuls (DoubleRow over k-tile pairs) as soon as
                        # both tiles of a pair are fully packed
                        zrows = 154 * (pair + 1)
                        while (
                            d_state["kt"] < NKT
                            and (128 * (d_state["kt"] + 2) <= zrows or pair == 7)
                        ):
                            kt = d_state["kt"]
                            nc.tensor.matmul(
                                d_state["ps"],
                                etd_sb[:, ds(kt, 2), :],
                                zb[:, ds(kt, 2), :],
                                start=(kt == 0),
                                stop=(kt == NKT - 2),
                                perf_mode=DR,
                            )
                            d_state["kt"] += 2

                    def emit_dinv(d_state):
                        dinv = soft.tile([16, NQ], F32, tag="dinv", bufs=2)
                        nc.vector.reciprocal_approx_fast(dinv, d_state["ps"][ds(0, 16), :])
                        dinv_bf = soft.tile([49, NQ], BF16, tag="dinvbf", bufs=2)
                        nc.any.memset(dinv_bf, 1.0)
                        nc.scalar.activation(
                            dinv_bf[ds(0, 16), :], dinv, AF.Copy, scale=PSC
                        )
                        nc.scalar.activation(
                            dinv_bf[ds(32, 16), :], dinv, AF.Copy, scale=PSC
                        )
                        return dinv_bf

                    def expand_norm(ci, dinv_bf):
                        zb = z_bufs[ci % 2]
                        pb = prob_bufs[ci % 2]
                        for kt in range(NKT):
                            # alternate row-strips 0/1 so consecutive expand
                            # matmuls overlap in the PE array
                            po = 32 * (kt % 2)
                            dexp_ps = ops.tile([128, NQ], F32, tag="dexp", bufs=2)
                            nc.tensor.matmul(
                                dexp_ps,
                                ex_sb[ds(po, 17), kt, :],
                                dinv_bf[ds(po, 17), :],
                                start=True,
                                stop=True,
                            )
                            nc.vector.tensor_mul(pb[:, kt, :], zb[:, kt, :], dexp_ps)

                    def av_group(ci, g):
                        q0 = NQ * ci
                        qj, nh = g // 2, g % 2
                        pb = prob_bufs[ci % 2]
                        res_t = st[ci]["res"]
                        o_ps = ops.tile([128, 512], F32, tag="ops", bufs=2)
                        for p5 in range(NKT // 2):
                            nc.tensor.matmul(
                                o_ps,
                                pb[:, ds(2 * p5, 2), ds(128 * qj, 128)],
                                m_f8[:, ds(2 * p5, 2), ds(512 * nh, 512)],
                                start=(p5 == 0),
                                stop=(p5 == NKT // 2 - 1),
                                perf_mode=DR,
                            )
                        if nh == 0:
                            st[ci][f"osb{qj}"] = work.tile(
                                [128, C], BF16, tag="osb", bufs=3, name=f"osb{ci}_{qj}"
                            )
                        o_sb = st[ci][f"osb{qj}"]
                        nc.vector.scalar_tensor_tensor(
                            o_sb[:, ds(512 * nh, 512)],
                            o_ps,
                            1.0 / (PSC * MSC),
                            res_t[:, qj, ds(512 * nh, 512)],
                            op0=ALU.mult,
                            op1=ALU.add,
                        )
                        if nh == 1:
                            nc.sync.dma_start(
                                out[ds(q0 + 128 * qj, 128), :], o_sb
                            )

                    for ci in range(NCHUNK):
                        if ci + 1 < NCHUNK:
                            load(ci + 1)
                        d_state = {
                            "kt": 0,
                            "ps": ops.tile([32, NQ], F32, tag="dps", bufs=1, name="dps"),
                        }
                        euT = chunk_eu(ci)
                        for pair in range(H // 2):
                            if ci > 0:
                                av_group(ci - 1, pair)
                            sm_pair(ci, pair, d_state, euT)
                            if pair < 6 and ci + 1 < NCHUNK:
                                qt_group(ci + 1, pair, ops)
                        dinv_bf = emit_dinv(d_state)
                        if ci + 1 < NCHUNK:
                            qt_group(ci + 1, 6, ops)
                            qt_group(ci + 1, 7, ops)
                        expand_norm(ci, dinv_bf)
                    for g in range(8):
                        av_group(NCHUNK - 1, g)

    nc.compile()
    return nc


_NC_CACHE = {}


def get_nc():
    if "nc" not in _NC_CACHE:
        _NC_CACHE["nc"] = build_nc()
    return _NC_CACHE["nc"]


def _bf16(x):
    return np.asarray(x, dtype=ml_dtypes.bfloat16)


def _fp8(x):
    return np.clip(np.asarray(x, np.float32), -240.0, 240.0).astype(
        ml_dtypes.float8_e4m3
    )


def make_in_maps(inputs):
    hs = np.asarray(inputs["hidden_states"], dtype=np.float32)
    ehs = np.asarray(inputs["encoder_hidden_states"], dtype=np.float32)
    mask_A = np.asarray(inputs["mask_A"], dtype=np.float32)
    mask_B = np.asarray(inputs["mask_B"], dtype=np.float32)
    Wq = np.asarray(inputs["Wq"], dtype=np.float32)
    Wk = np.asarray(inputs["Wk"], dtype=np.float32)
    Wv = np.asarray(inputs["Wv"], dtype=np.float32)
    Wo = np.asarray(inputs["Wo"], dtype=np.float32)
    bo = np.asarray(inputs["bo"], dtype=np.float32)
    idxA = np.asarray(inputs["token_indices_A"]).astype(np.int64) % T
    idxB = np.asarray(inputs["token_indices_B"]).astype(np.int64) % T

    # rank-2 suppression: bias[t,q] = ind2[:,t] . mask2[:,q], with B-set
    # overwriting A-set (reference applies A then B)
    inA = np.zeros(T, np.float32)
    inA[idxA] = 1.0
    inB = np.zeros(T, np.float32)
    inB[idxB] = 1.0
    ind2_np = np.stack([-SUPPRESS * inA * (1.0 - inB), -SUPPRESS * inB])
    mask2_np = np.stack([1.0 - mask_A, 1.0 - mask_B])

    # D-sum indicator [p, kt*16+h] and expand indicator [h(17), kt*128+p]
    rows = np.arange(NKT * 128)
    head_of = np.where(rows < H * T, rows // T, -1)
    etd_np = np.zeros((128, NKT * 32), np.float32)
    ex_np = np.zeros((17, NKT * 128), np.float32)
    for kt in range(NKT):
        for p in range(128):
            hh = head_of[kt * 128 + p]
            if 0 <= hh < H:
                etd_np[p, kt * 32 + hh] = 1.0
                ex_np[hh, kt * 128 + p] = 1.0
    ex_np[16, BO_TILE * 128 + BO_PART] = 1.0

    wq_f8 = _fp8(Wq * (QSC / np.sqrt(D)))
    wk_f8, wv_f8 = _fp8(Wk * 64.0), _fp8(Wv * 64.0)
    wo_bf = _bf16(Wo * MSC)
    bo_f8 = _fp8(bo * MSC)[None, :]
    ind2_bf, mask2_bf = _bf16(ind2_np), _bf16(mask2_np)
    etd_bf, ex_bf = _fp8(etd_np), _bf16(ex_np)

    in_maps = []
    for b in range(B):
        in_maps.append(
            {
                "hsT": _fp8(hs[b].T),
                "hsres": _bf16(hs[b]),
                "ehsT": _fp8(ehs[b].T.copy()),
                "wq": wq_f8,
                "wk": wk_f8,
                "wv": wv_f8,
                "wo": wo_bf,
                "bo": bo_f8,
                "ind2": ind2_bf,
                "mask2": mask2_bf,
                "etd": etd_bf,
                "exp_ind": ex_bf,
            }
        )
    return in_maps


def kernel(**inputs) -> np.ndarray:
    from concourse.bass_utils import run_bass_kernel_spmd

    nc = get_nc()
    in_maps = make_in_maps(inputs)
    res = run_bass_kernel_spmd(nc, in_maps, core_ids=list(range(B)))
    return np.stack([res.results[b]["out"] for b in range(B)]).astype(np.float32)


# revision 21
# speedup vs baseline: 1.1688x; 1.0676x over previous
"""Trainium2 Bass kernel for nn_DenseAttnProcessor (sparse_attention).

Cross-attention block: q = hs@Wq, k/v = ehs@{Wk,Wv}, per-head softmax((q k^T)/8
+ col_bias) @ v, @Wo + bo + residual.  B=8 batches -> data-parallel, one batch
per NeuronCore (no collectives).

v2 dataflow (per core):

  host prep:  hsT fp8 (pre-transposed, so no runtime DMA-transpose), hs
              residual bf16, Wq*8 fp8, Wo*16 bf16, suppression bias factored
              rank-2: ind2 [2,77] (x -SUPPRESS) and mask2 [2,HW] so that
              col_bias^T = ind2^T @ mask2 with exact set-overwrite semantics.
  stage A:    k,v = ehsT^T @ {Wk,Wv} (bf16); kT = PE-transpose(k)/64;
              M_h = v_h @ (16 Wo_h); M rows packed fp8 into [128,10,1024]
              stacked tiles (+ 16*bo row at stacked row 1232).
  stage B (8 chunks of 512 q rows, software-pipelined):
    qT   = (8Wq)^T @ hsT   -- fp8 DoubleRow, 2 k-tiles per matmul
    per head pair: scoresT [77,512] = kT_h^T qT_h (K=64) accumulated with
         the K=2 rank-2 suppression matmul (ind2 x mask2 chunk)
    z    = Exp(scoresT) on Scalar -> DMA-packed into stacked [128,10,512]
    D    = batched over all heads: 10 indicator matmuls -> [16,512] psum
    dinv = 32/D (fast DVE reciprocal, x32 folded at the bf16 copy)
    dexp = indicator-expand matmul back to [128,512] per k-tile
    prob = z * dexp  (DVE, fp8 out, = 32*softmax)
    AV   = prob^T @ M  -- fp8 DoubleRow, 5 matmuls per [128,512] psum
    out  = psum/512 + residual (fused DVE scalar_tensor_tensor), bf16 store.

AV of chunk ci is interleaved into the head loop of chunk ci+1 to keep the PE
stream dense (HAM stays warm).
"""

import sys

for _p in ("/opt/trn_rl_repo",):
    if _p not in sys.path:
        sys.path.insert(0, _p)

import numpy as np
import ml_dtypes

import concourse.mybir as mybir
import concourse.tile as tile
from concourse import bacc
from concourse.bass import ds
from concourse.masks import make_identity

F32 = mybir.dt.float32
BF16 = mybir.dt.bfloat16
FP8 = mybir.dt.float8e4
AF = mybir.ActivationFunctionType
ALU = mybir.AluOpType
DR = mybir.MatmulPerfMode.DoubleRow

B, HW, C, CT, T, H, D = 8, 4096, 1024, 2048, 77, 16, 64
SUPPRESS = 20.0
RT = H * T + 1                # 1233 stacked rows (16*77 head rows + bo row)
NKT = (RT + 127) // 128       # 10 K-tiles for the AV matmul
NQ = 512                      # q rows per chunk
NCHUNK = HW // NQ             # 8
BO_TILE, BO_PART = (H * T) // 128, (H * T) % 128   # bo/ones row: tile 9, p 80
PSC = 32.0                    # probs scale (fp8 range)
MSC = 16.0                    # M scale (fp8 range)
QSC = 64.0                    # q scale (Wq*scale*64 fp8, kT/64)


def _pack_pieces(h):
    """DMA pieces for packing head h's 77 rows at stacked row 77*h, split at
    128-row tile boundaries.  Returns list of (tile_idx, part_base, src_start,
    nrows)."""
    g = T * h
    pieces = []
    pos = 0
    while pos < T:
        gg = g + pos
        ti, d = gg // 128, gg % 128
        n = min(T - pos, 128 - d)
        pieces.append((ti, d, pos, n))
        pos += n
    return pieces


def build_nc():
    nc = bacc.Bacc("TRN2", target_bir_lowering=False, debug=False)

    hsT = nc.dram_tensor("hsT", [128, C // 128, HW], FP8, kind="ExternalInput")
    hsres = nc.dram_tensor("hsres", [128, HW // 128, C], BF16, kind="ExternalInput")
    ehsT = nc.dram_tensor("ehsT", [128, CT // 128, 80], FP8, kind="ExternalInput")
    wq = nc.dram_tensor("wq", [128, C // 128, C], FP8, kind="ExternalInput")
    wk = nc.dram_tensor("wk", [128, CT // 128, C], FP8, kind="ExternalInput")
    wv = nc.dram_tensor("wv", [128, CT // 128, C], FP8, kind="ExternalInput")
    wo = nc.dram_tensor("wo", [128, C // 128, C], BF16, kind="ExternalInput")
    bo = nc.dram_tensor("bo", [1, C], FP8, kind="ExternalInput")
    ind2 = nc.dram_tensor("ind2", [2, T], BF16, kind="ExternalInput")
    mask2 = nc.dram_tensor("mask2", [2, HW], BF16, kind="ExternalInput")
    etd = nc.dram_tensor("etd", [128, NKT * 32], FP8, kind="ExternalInput")
    exp_ind = nc.dram_tensor("exp_ind", [17, NKT * 128], BF16, kind="ExternalInput")
    out = nc.dram_tensor("out", [128, HW // 128, C], BF16, kind="ExternalOutput")

    with tile.TileContext(nc) as tc:
        with (
            tc.tile_pool(name="const", bufs=1) as const,
            tc.tile_pool(name="persist", bufs=1) as persist,
        ):
            ident = const.tile([128, 128], BF16)
            make_identity(nc, ident)
            ind2_sb = const.tile([2, T], BF16)
            nc.sync.dma_start(ind2_sb, ind2[:, :])
            mask2_sb = const.tile([2, HW], BF16)
            nc.sync.dma_start(mask2_sb, mask2[:, :])
            etd_sb = const.tile([128, NKT, 32], FP8)
            nc.sync.dma_start(etd_sb, etd[:, :])
            ex_sb = const.tile([49, NKT, 128], BF16)
            nc.sync.dma_start(ex_sb[ds(0, 17), :, :], exp_ind[:, :])
            nc.sync.dma_start(ex_sb[ds(32, 17), :, :], exp_ind[:, :])

            # persistent stacks
            kT_sb = persist.tile([128, C // 128, T], BF16)        # [inner, t]
            m_f8 = persist.tile([128, NKT, C], FP8)               # stacked 16*M
            wq_sb = persist.tile([128, C // 128, C], FP8)
            nc.sync.dma_start(wq_sb, wq[:, :, :])
            z_bufs = [persist.tile([128, NKT, NQ], FP8, name=f"z{b}") for b in range(2)]
            prob_bufs = [persist.tile([128, NKT, NQ], FP8, name=f"pb{b}") for b in range(2)]
            psc_row = const.tile([1, NQ], FP8)
            nc.any.memset(psc_row, PSC)
            for zb in z_bufs:
                # bo/ones pseudo-row = PSC; rows past it zero (NaN hygiene for
                # the D matmul which reads all 128 partitions).  memset can
                # only start at 32-aligned partitions; DMA patches row 80.
                nc.any.memset(zb[ds(64, 64), BO_TILE, :], 0.0)
                nc.sync.dma_start(zb[ds(BO_PART, 1), BO_TILE, :], psc_row)
            # M stack tile 9: rows past head rows; bo*16 at BO_PART
            nc.any.memset(m_f8[ds(64, 64), BO_TILE, :], 0.0)
            nc.sync.dma_start(m_f8[ds(BO_PART, 1), BO_TILE, :], bo[:, :])

            st = {}

            with (
                tc.tile_pool(name="hsp", bufs=2) as hsp,
                tc.tile_pool(name="work", bufs=2) as work,
                tc.tile_pool(name="soft", bufs=4) as soft,
            ):

                def load(ci):
                    q0 = NQ * ci
                    hsT_t = hsp.tile([128, C // 128, NQ], FP8, tag="hsT")
                    nc.sync.dma_start(hsT_t, hsT[:, :, ds(q0, NQ)])
                    res_t = hsp.tile([128, NQ // 128, C], BF16, tag="res", bufs=3)
                    nc.sync.dma_start(
                        res_t, hsres[:, ds(ci * (NQ // 128), NQ // 128), :]
                    )
                    qT = work.tile([128, C // 128, NQ], BF16, tag="qT")
                    st[ci] = dict(hsT=hsT_t, res=res_t, qT=qT)

                def qt_group(ci, ij, ps_pool, tag="qps", bufs=1):
                    hsT_t, qT = st[ci]["hsT"], st[ci]["qT"]
                    q_ps = ps_pool.tile([128, NQ], F32, tag=tag, bufs=bufs)
                    for c2 in range(C // 256):
                        nc.tensor.matmul(
                            q_ps,
                            wq_sb[:, ds(2 * c2, 2), ds(128 * ij, 128)],
                            hsT_t[:, ds(2 * c2, 2), :],
                            start=(c2 == 0),
                            stop=(c2 == C // 256 - 1),
                            perf_mode=DR,
                        )
                    nc.any.tensor_copy(qT[:, ij, :], q_ps)

                # ---------------- stage A: k, v, kT, M ----------------
                with (
                    tc.tile_pool(name="sa_sb", bufs=3) as sa_sb,
                    tc.tile_pool(name="sa_w", bufs=4) as sa_w,
                    tc.tile_pool(name="sa_ps", bufs=2, space="PSUM") as sa_ps,
                ):
                    # chunk-0 loads + qT(0) early so PE/DMA warm up while the
                    # k/v weight tiles stream in.  "big" [128,1024] psum tag is
                    # shared by qT(0) (first 512 cols) and the M matmuls.
                    load(0)
                    ehsT_sb = sa_sb.tile([128, CT // 128, 80], FP8, bufs=1)
                    nc.sync.dma_start(ehsT_sb, ehsT[:, :, :])

                    def big_ps():
                        return sa_ps.tile([128, C], F32, tag="big", bufs=2, name="bigps")

                    for ij in range(C // 128):
                        hsT_t, qT = st[0]["hsT"], st[0]["qT"]
                        q_ps = big_ps()
                        for c2 in range(C // 256):
                            nc.tensor.matmul(
                                q_ps[:, ds(0, NQ)],
                                wq_sb[:, ds(2 * c2, 2), ds(128 * ij, 128)],
                                hsT_t[:, ds(2 * c2, 2), :],
                                start=(c2 == 0),
                                stop=(c2 == C // 256 - 1),
                                perf_mode=DR,
                            )
                        nc.any.tensor_copy(qT[:, ij, :], q_ps[:, ds(0, NQ)])

                    kv_sb = {}
                    for name, wten in (("k", wk), ("v", wv)):
                        kv_ps = sa_ps.tile([80, C], F32, tag="kvps", bufs=1)
                        wt = sa_w.tile([128, CT // 128, C], FP8, tag=f"w{name}", bufs=1)
                        nc.sync.dma_start(wt, wten[:, :, :])
                        for nh in range(2):
                            for j2 in range(CT // 256):
                                nc.tensor.matmul(
                                    kv_ps[:, ds(512 * nh, 512)],
                                    ehsT_sb[:, ds(2 * j2, 2), :],
                                    wt[:, ds(2 * j2, 2), ds(512 * nh, 512)],
                                    start=(j2 == 0),
                                    stop=(j2 == CT // 256 - 1),
                                    perf_mode=DR,
                                )
                        kvs = sa_sb.tile([T, C], BF16, tag=f"{name}sb", bufs=1)
                        # host scales Wk/Wv by 64 for fp8 range; fold out here
                        # (k also folds 1/QSC so scoresT = (k/64)^T (64 q/8))
                        sc = 1.0 / (QSC * 64.0) if name == "k" else 1.0 / 64.0
                        nc.scalar.activation(kvs, kv_ps[ds(0, T), :], AF.Copy, scale=sc)
                        kv_sb[name] = kvs

                    # kT / vT via PE transpose of 128-column slices
                    vT_sb = sa_sb.tile([128, C // 128, T], BF16, bufs=1)
                    for src, dst in ((kv_sb["k"], kT_sb), (kv_sb["v"], vT_sb)):
                        for i in range(C // 128):
                            tp = sa_ps.tile([128, T], BF16, tag="tpa")
                            nc.tensor.transpose(tp, src[:, ds(128 * i, 128)], ident[:T, :T])
                            nc.any.tensor_copy(dst[:, i, :], tp)

                    # M_h = v_h @ (16 Wo_h), fp8-packed at stacked row 77h.
                    # Head pairs run in disjoint row-strips on two psum banks.
                    wot_all = sa_w.tile([128, C // 128, C], BF16, tag="wot", bufs=1)
                    nc.sync.dma_start(wot_all, wo[:, :, :])
                    for i in range(C // 128):
                        wot = wot_all[:, i, :]
                        mps = [big_ps(), big_ps()]
                        for nh in range(2):
                            for sub in range(2):
                                nc.tensor.matmul(
                                    mps[sub][ds(0, T), ds(512 * nh, 512)],
                                    vT_sb[ds(64 * sub, 64), i, :],
                                    wot[ds(64 * sub, 64), ds(512 * nh, 512)],
                                    start=True,
                                    stop=True,
                                )
                        for sub in range(2):
                            h = 2 * i + sub
                            m_stg = sa_sb.tile([T, C], FP8, tag="mstg")
                            nc.any.tensor_copy(m_stg, mps[sub][ds(0, T), :])
                            for (ti, pb, s0, nr) in _pack_pieces(h):
                                nc.gpsimd.dma_start(
                                    m_f8[ds(pb, nr), ti, :], m_stg[ds(s0, nr), :]
                                )

                # ---------------- stage B ----------------
                with tc.tile_pool(name="ops", bufs=2, space="PSUM") as ops:

                    def chunk_eu(ci):
                        # shared suppression factor exp(col_bias)^T for this
                        # chunk: one K=2 matmul + one Exp, used by all heads
                        q0 = NQ * ci
                        b_ps = ops.tile([T, NQ], F32, tag="sT", bufs=2, name="bps")
                        nc.tensor.matmul(
                            b_ps, ind2_sb, mask2_sb[:, ds(q0, NQ)], start=True, stop=True
                        )
                        euT = soft.tile([T, NQ], BF16, tag="euT", bufs=2)
                        nc.scalar.activation(euT, b_ps, AF.Exp)
                        return euT

                    def sm_pair(ci, pair, d_state, euT):
                        qT = st[ci]["qT"]
                        zb = z_bufs[ci % 2]
                        sps = []
                        for sub in range(2):
                            po = 64 * sub
                            sT_ps = ops.tile([T, NQ], F32, tag="sT", bufs=2)
                            nc.tensor.matmul(
                                sT_ps,
                                kT_sb[ds(po, 64), pair, :],
                                qT[ds(po, 64), pair, :],
                                start=True,
                                stop=True,
                            )
                            sps.append(sT_ps)
                        for sub in range(2):
                            h = 2 * pair + sub
                            z_h = soft.tile([T, NQ], FP8, tag="zh", bufs=4)
                            nc.scalar.activation(z_h, sps[sub], AF.Exp)
                            z2 = soft.tile([T, NQ], FP8, tag="z2", bufs=4)
                            nc.vector.tensor_mul(z2, z_h, euT)
                            for (ti, pb, s0, nr) in _pack_pieces(h):
                                nc.gpsimd.dma_start(
                                    zb[ds(pb, nr), ti, :], z2[ds(s0, nr), :]
                                )
                        # D matmuls (DoubleRow over k-tile pairs) as soon as
                        # both tiles of a pair are fully packed
                        zrows = 154 * (pair + 1)
                        while (
                            d_state["kt"] < NKT
                            and (128 * (d_state["kt"] + 2) <= zrows or pair == 7)
                        ):
                            kt = d_state["kt"]
                            nc.tensor.matmul(
                                d_state["ps"],
                                etd_sb[:, ds(kt, 2), :],
                                zb[:, ds(kt, 2), :],
                                start=(kt == 0),
                                stop=(kt == NKT - 2),
                                perf_mode=DR,
                            )
                            d_state["kt"] += 2

                    def emit_dinv(d_state):
                        dinv = soft.tile([16, NQ], F32, tag="dinv", bufs=2)
                        nc.vector.reciprocal_approx_fast(dinv, d_state["ps"][ds(0, 16), :])
                        dinv_bf = soft.tile([49, NQ], BF16, tag="dinvbf", bufs=2)
                        nc.any.memset(dinv_bf, 1.0)
                        nc.scalar.activation(
                            dinv_bf[ds(0, 16), :], dinv, AF.Copy, scale=PSC
                        )
                        nc.scalar.activation(
                            dinv_bf[ds(32, 16), :], dinv, AF.Copy, scale=PSC
                        )
                        return dinv_bf

                    def expand_norm(ci, dinv_bf):
                        zb = z_bufs[ci % 2]
                        pb = prob_bufs[ci % 2]
                        for kt in range(NKT):
                            # alternate row-strips 0/1 so consecutive expand
                            # matmuls overlap in the PE array
                            po = 32 * (kt % 2)
                            dexp_ps = ops.tile([128, NQ], F32, tag="dexp", bufs=2)
                            nc.tensor.matmul(
                                dexp_ps,
                                ex_sb[ds(po, 17), kt, :],
                                dinv_bf[ds(po, 17), :],
                                start=True,
                                stop=True,
                            )
                            nc.vector.tensor_mul(pb[:, kt, :], zb[:, kt, :], dexp_ps)

                    def av_group(ci, g):
                        qj, nh = g // 2, g % 2
                        pb = prob_bufs[ci % 2]
                        res_t = st[ci]["res"]
                        o_ps = ops.tile([128, 512], F32, tag="ops", bufs=2)
                        for p5 in range(NKT // 2):
                            nc.tensor.matmul(
                                o_ps,
                                pb[:, ds(2 * p5, 2), ds(128 * qj, 128)],
                                m_f8[:, ds(2 * p5, 2), ds(512 * nh, 512)],
                                start=(p5 == 0),
                                stop=(p5 == NKT // 2 - 1),
                                perf_mode=DR,
                            )
                        if g == 0:
                            st[ci]["osb"] = work.tile(
                                [128, NQ // 128, C], BF16, tag="osb", bufs=2,
                                name=f"osb{ci}",
                            )
                        o_sb = st[ci]["osb"]
                        nc.vector.scalar_tensor_tensor(
                            o_sb[:, qj, ds(512 * nh, 512)],
                            o_ps,
                            1.0 / (PSC * MSC),
                            res_t[:, qj, ds(512 * nh, 512)],
                            op0=ALU.mult,
                            op1=ALU.add,
                        )
                        if g == 7:
                            nc.sync.dma_start(
                                out[:, ds(ci * (NQ // 128), NQ // 128), :], o_sb
                            )

                    for ci in range(NCHUNK):
                        if ci + 1 < NCHUNK:
                            load(ci + 1)
                        d_state = {
                            "kt": 0,
                            "ps": ops.tile([32, NQ], F32, tag="dps", bufs=1, name="dps"),
                        }
                        euT = chunk_eu(ci)
                        for pair in range(H // 2):
                            if ci > 0:
                                av_group(ci - 1, pair)
                            sm_pair(ci, pair, d_state, euT)
                            if pair < 6 and ci + 1 < NCHUNK:
                                qt_group(ci + 1, pair, ops)
                        dinv_bf = emit_dinv(d_state)
                        if ci + 1 < NCHUNK:
                            qt_group(ci + 1, 6, ops)
                            qt_group(ci + 1, 7, ops)
                        expand_norm(ci, dinv_bf)
                    for g in range(8):
                        av_group(NCHUNK - 1, g)

    nc.compile()
    return nc


_NC_CACHE = {}


def get_nc():
    if "nc" not in _NC_CACHE:
        _NC_CACHE["nc"] = build_nc()
    return _NC_CACHE["nc"]


def _bf16(x):
    return np.asarray(x, dtype=ml_dtypes.bfloat16)


def _fp8(x):
    return np.clip(np.asarray(x, np.float32), -240.0, 240.0).astype(
        ml_dtypes.float8_e4m3
    )


def make_in_maps(inputs):
    hs = np.asarray(inputs["hidden_states"], dtype=np.float32)
    ehs = np.asarray(inputs["encoder_hidden_states"], dtype=np.float32)
    mask_A = np.asarray(inputs["mask_A"], dtype=np.float32)
    mask_B = np.asarray(inputs["mask_B"], dtype=np.float32)
    Wq = np.asarray(inputs["Wq"], dtype=np.float32)
    Wk = np.asarray(inputs["Wk"], dtype=np.float32)
    Wv = np.asarray(inputs["Wv"], dtype=np.float32)
    Wo = np.asarray(inputs["Wo"], dtype=np.float32)
    bo = np.asarray(inputs["bo"], dtype=np.float32)
    idxA = np.asarray(inputs["token_indices_A"]).astype(np.int64) % T
    idxB = np.asarray(inputs["token_indices_B"]).astype(np.int64) % T

    # rank-2 suppression: bias[t,q] = ind2[:,t] . mask2[:,q], with B-set
    # overwriting A-set (reference applies A then B)
    inA = np.zeros(T, np.float32)
    inA[idxA] = 1.0
    inB = np.zeros(T, np.float32)
    inB[idxB] = 1.0
    ind2_np = np.stack([-SUPPRESS * inA * (1.0 - inB), -SUPPRESS * inB])
    mask2_np = np.stack([1.0 - mask_A, 1.0 - mask_B])

    # D-sum indicator [p, kt*16+h] and expand indicator [h(17), kt*128+p]
    rows = np.arange(NKT * 128)
    head_of = np.where(rows < H * T, rows // T, -1)
    etd_np = np.zeros((128, NKT * 32), np.float32)
    ex_np = np.zeros((17, NKT * 128), np.float32)
    for kt in range(NKT):
        for p in range(128):
            hh = head_of[kt * 128 + p]
            if 0 <= hh < H:
                etd_np[p, kt * 32 + hh] = 1.0
                ex_np[hh, kt * 128 + p] = 1.0
    ex_np[16, BO_TILE * 128 + BO_PART] = 1.0

    def _tile3(a):
        # [R, C] -> [128, R//128, C] with row r = 128*j + p at [p, j, :]
        R = a.shape[0]
        return np.ascontiguousarray(
            a.reshape(R // 128, 128, a.shape[1]).transpose(1, 0, 2)
        )

    wq_f8 = _fp8(_tile3(Wq * (QSC / np.sqrt(D))))
    wk_f8, wv_f8 = _fp8(_tile3(Wk * 64.0)), _fp8(_tile3(Wv * 64.0))
    wo_bf = _bf16(_tile3(Wo * MSC))
    bo_f8 = _fp8(bo * MSC)[None, :]
    ind2_bf, mask2_bf = _bf16(ind2_np), _bf16(mask2_np)
    etd_bf, ex_bf = _fp8(etd_np), _bf16(ex_np)

    ehsT_f8 = np.zeros((B, 128, CT // 128, 80), ml_dtypes.float8_e4m3)
    for b in range(B):
        ehsT_f8[b, :, :, :T] = _fp8(_tile3(ehs[b].T.copy()))

    in_maps = []
    for b in range(B):
        in_maps.append(
            {
                "hsT": _fp8(_tile3(hs[b].T)),
                "hsres": _bf16(_tile3(hs[b])),
                "ehsT": ehsT_f8[b],
                "wq": wq_f8,
                "wk": wk_f8,
                "wv": wv_f8,
                "wo": wo_bf,
                "bo": bo_f8,
                "ind2": ind2_bf,
                "mask2": mask2_bf,
                "etd": etd_bf,
                "exp_ind": ex_bf,
            }
        )
    return in_maps


def kernel(**inputs) -> np.ndarray:
    from concourse.bass_utils import run_bass_kernel_spmd

    nc = get_nc()
    in_maps = make_in_maps(inputs)
    res = run_bass_kernel_spmd(nc, in_maps, core_ids=list(range(B)))
    outs = []
    for b in range(B):
        o = np.asarray(res.results[b]["out"])  # [128, 32, 1024]
        outs.append(o.transpose(1, 0, 2).reshape(HW, C))
    return np.stack(outs).astype(np.float32)


# revision 23
# speedup vs baseline: 1.2185x; 1.0425x over previous
"""Trainium2 Bass kernel for nn_DenseAttnProcessor (sparse_attention).

Cross-attention block: q = hs@Wq, k/v = ehs@{Wk,Wv}, per-head softmax((q k^T)/8
+ col_bias) @ v, @Wo + bo + residual.  B=8 batches -> data-parallel, one batch
per NeuronCore (no collectives).  716us baseline -> 418us.

Per-core dataflow:

  host prep:  all operands pre-tiled to [128, ntiles, free] so every load is
              ONE DMA (the ~680ns per-dma issue cost on the Sync sequencer was
              a bottleneck); hs pre-transposed to fp8 hsT (no runtime
              DMA-transpose); residual bf16; Wq*8 fp8, Wk/Wv*64 fp8, Wo*16
              bf16; suppression bias factored rank-2 (ind2 [2,77] x mask2
              [2,HW], exact set-overwrite semantics); head-indicator matrices
              for the batched softmax denominator.
  stage A:    k,v = ehsT^T @ {Wk,Wv} (fp8 DoubleRow, 2 k-tiles/matmul);
              kT = PE-transpose(k)/4096; M_h = v_h @ (16 Wo_h) (head pairs in
              disjoint array row-strips overlap); M packed fp8 into
              [128,10,1024] stacked tiles (+16*bo row at stacked row 1232).
  stage B (8 chunks of 512 q rows, software-pipelined; AV of chunk ci runs
  inside the head loop of chunk ci+1 so the PE stream stays dense):
    euT  = Exp(ind2^T @ mask2-chunk)  -- one K=2 matmul + Exp per chunk
    qT   = (8Wq)^T @ hsT              -- fp8 DoubleRow
    per head pair: scoresT [77,512] = kT_h^T qT_h (K=64; the pair runs in
         disjoint row-strips / psum banks and overlaps)
    z    = Exp(scoresT) fp8 * euT (DVE) -> packed into stacked [128,10,512]
         (gpsimd dma), D matmuls chase the packs incrementally
    D    = indicator^T @ zstack       -- 5 fp8-DoubleRow matmuls, [16,512]
    dinv = 32/D (fast DVE reciprocal)
    dexp = expand-indicator^T @ dinv per k-tile (alternating row-strips)
    prob = z * dexp  (DVE, fp8, = 32*softmax)
    AV   = prob^T @ M                 -- fp8 DoubleRow, 5 matmuls per psum
    out  = psum/512 + residual (fused DVE scalar_tensor_tensor), one bf16
           store per chunk in pre-tiled layout (host untiles).
"""

import sys

for _p in ("/opt/trn_rl_repo",):
    if _p not in sys.path:
        sys.path.insert(0, _p)

import numpy as np
import ml_dtypes

import concourse.mybir as mybir
import concourse.tile as tile
from concourse import bacc
from concourse.bass import ds
from concourse.masks import make_identity

F32 = mybir.dt.float32
BF16 = mybir.dt.bfloat16
FP8 = mybir.dt.float8e4
AF = mybir.ActivationFunctionType
ALU = mybir.AluOpType
DR = mybir.MatmulPerfMode.DoubleRow

B, HW, C, CT, T, H, D = 8, 4096, 1024, 2048, 77, 16, 64
SUPPRESS = 20.0
RT = H * T + 1                # 1233 stacked rows (16*77 head rows + bo row)
NKT = (RT + 127) // 128       # 10 K-tiles for the AV matmul
NQ = 512                      # q rows per chunk
NCHUNK = HW // NQ             # 8
BO_TILE, BO_PART = (H * T) // 128, (H * T) % 128   # bo/ones row: tile 9, p 80
PSC = 32.0                    # probs scale (fp8 range)
MSC = 16.0                    # M scale (fp8 range)
QSC = 64.0                    # q scale (Wq*scale*64 fp8, kT/64)


def _pack_pieces(h):
    """DMA pieces for packing head h's 77 rows at stacked row 77*h, split at
    128-row tile boundaries.  Returns list of (tile_idx, part_base, src_start,
    nrows)."""
    g = T * h
    pieces = []
    pos = 0
    while pos < T:
        gg = g + pos
        ti, d = gg // 128, gg % 128
        n = min(T - pos, 128 - d)
        pieces.append((ti, d, pos, n))
        pos += n
    return pieces


def build_nc():
    nc = bacc.Bacc("TRN2", target_bir_lowering=False, debug=False)

    hsT = nc.dram_tensor("hsT", [128, C // 128, HW], FP8, kind="ExternalInput")
    hsres = nc.dram_tensor("hsres", [128, HW // 128, C], BF16, kind="ExternalInput")
    ehsT = nc.dram_tensor("ehsT", [128, CT // 128, 80], FP8, kind="ExternalInput")
    wq = nc.dram_tensor("wq", [128, C // 128, C], FP8, kind="ExternalInput")
    wk = nc.dram_tensor("wk", [128, CT // 128, C], FP8, kind="ExternalInput")
    wv = nc.dram_tensor("wv", [128, CT // 128, C], FP8, kind="ExternalInput")
    wo = nc.dram_tensor("wo", [128, C // 128, C], BF16, kind="ExternalInput")
    bo = nc.dram_tensor("bo", [1, C], FP8, kind="ExternalInput")
    ind2 = nc.dram_tensor("ind2", [2, T], BF16, kind="ExternalInput")
    mask2 = nc.dram_tensor("mask2", [2, HW], BF16, kind="ExternalInput")
    etd = nc.dram_tensor("etd", [128, NKT * 32], FP8, kind="ExternalInput")
    exp_ind = nc.dram_tensor("exp_ind", [17, NKT * 128], BF16, kind="ExternalInput")
    out = nc.dram_tensor("out", [128, HW // 128, C], BF16, kind="ExternalOutput")

    with tile.TileContext(nc) as tc:
        with (
            tc.tile_pool(name="const", bufs=1) as const,
            tc.tile_pool(name="persist", bufs=1) as persist,
        ):
            ident = const.tile([128, 128], BF16)
            make_identity(nc, ident)
            ind2_sb = const.tile([2, T], BF16)
            nc.sync.dma_start(ind2_sb, ind2[:, :])
            mask2_sb = const.tile([2, HW], BF16)
            nc.sync.dma_start(mask2_sb, mask2[:, :])
            etd_sb = const.tile([128, NKT, 32], FP8)
            nc.sync.dma_start(etd_sb, etd[:, :])
            ex_sb = const.tile([49, NKT, 128], BF16)
            nc.sync.dma_start(ex_sb[ds(0, 17), :, :], exp_ind[:, :])
            nc.sync.dma_start(ex_sb[ds(32, 17), :, :], exp_ind[:, :])

            # persistent stacks
            kT_sb = persist.tile([128, C // 128, T], BF16)        # [inner, t]
            m_f8 = persist.tile([128, NKT, C], FP8)               # stacked 16*M
            wq_sb = persist.tile([128, C // 128, C], FP8)
            nc.sync.dma_start(wq_sb, wq[:, :, :])
            z_bufs = [persist.tile([128, NKT, NQ], FP8, name=f"z{b}") for b in range(2)]
            prob_bufs = [persist.tile([128, NKT, NQ], FP8, name=f"pb{b}") for b in range(2)]
            psc_row = const.tile([1, NQ], FP8)
            nc.any.memset(psc_row, PSC)
            for zb in z_bufs:
                # bo/ones pseudo-row = PSC; rows past it zero (NaN hygiene for
                # the D matmul which reads all 128 partitions).  memset can
                # only start at 32-aligned partitions; DMA patches row 80.
                nc.any.memset(zb[ds(64, 64), BO_TILE, :], 0.0)
                nc.sync.dma_start(zb[ds(BO_PART, 1), BO_TILE, :], psc_row)
            # M stack tile 9: rows past head rows; bo*16 at BO_PART
            nc.any.memset(m_f8[ds(64, 64), BO_TILE, :], 0.0)
            nc.sync.dma_start(m_f8[ds(BO_PART, 1), BO_TILE, :], bo[:, :])

            st = {}

            with (
                tc.tile_pool(name="hsp", bufs=2) as hsp,
                tc.tile_pool(name="work", bufs=2) as work,
                tc.tile_pool(name="soft", bufs=4) as soft,
            ):

                def load(ci):
                    q0 = NQ * ci
                    hsT_t = hsp.tile([128, C // 128, NQ], FP8, tag="hsT")
                    for hh in range(2):
                        nc.sync.dma_start(
                            hsT_t[:, ds(4 * hh, 4), :],
                            hsT[:, ds(4 * hh, 4), ds(q0, NQ)],
                        )
                    res_t = hsp.tile([128, NQ // 128, C], BF16, tag="res", bufs=3)
                    for hh in range(2):
                        nc.sync.dma_start(
                            res_t[:, ds(2 * hh, 2), :],
                            hsres[:, ds(ci * (NQ // 128) + 2 * hh, 2), :],
                        )
                    qT = work.tile([128, C // 128, NQ], BF16, tag="qT")
                    st[ci] = dict(hsT=hsT_t, res=res_t, qT=qT)

                def qt_group(ci, ij, ps_pool, tag="qps", bufs=1):
                    hsT_t, qT = st[ci]["hsT"], st[ci]["qT"]
                    q_ps = ps_pool.tile([128, NQ], F32, tag=tag, bufs=bufs)
                    for c2 in range(C // 256):
                        nc.tensor.matmul(
                            q_ps,
                            wq_sb[:, ds(2 * c2, 2), ds(128 * ij, 128)],
                            hsT_t[:, ds(2 * c2, 2), :],
                            start=(c2 == 0),
                            stop=(c2 == C // 256 - 1),
                            perf_mode=DR,
                        )
                    nc.any.tensor_copy(qT[:, ij, :], q_ps)

                # ---------------- stage A: k, v, kT, M ----------------
                with (
                    tc.tile_pool(name="sa_sb", bufs=3) as sa_sb,
                    tc.tile_pool(name="sa_w", bufs=4) as sa_w,
                    tc.tile_pool(name="sa_ps", bufs=2, space="PSUM") as sa_ps,
                ):
                    # chunk-0 loads + qT(0) early so PE/DMA warm up while the
                    # k/v weight tiles stream in.  "big" [128,1024] psum tag is
                    # shared by qT(0) (first 512 cols) and the M matmuls.
                    load(0)
                    ehsT_sb = sa_sb.tile([128, CT // 128, 80], FP8, bufs=1)
                    nc.sync.dma_start(ehsT_sb, ehsT[:, :, :])

                    def big_ps():
                        return sa_ps.tile([128, C], F32, tag="big", bufs=2, name="bigps")

                    for ij in range(C // 128):
                        hsT_t, qT = st[0]["hsT"], st[0]["qT"]
                        q_ps = big_ps()
                        for c2 in range(C // 256):
                            nc.tensor.matmul(
                                q_ps[:, ds(0, NQ)],
                                wq_sb[:, ds(2 * c2, 2), ds(128 * ij, 128)],
                                hsT_t[:, ds(2 * c2, 2), :],
                                start=(c2 == 0),
                                stop=(c2 == C // 256 - 1),
                                perf_mode=DR,
                            )
                        nc.any.tensor_copy(qT[:, ij, :], q_ps[:, ds(0, NQ)])

                    kv_sb = {}
                    for name, wten in (("k", wk), ("v", wv)):
                        kv_ps = sa_ps.tile([80, C], F32, tag="kvps", bufs=1)
                        wt = sa_w.tile([128, CT // 128, C], FP8, tag=f"w{name}", bufs=1)
                        nc.sync.dma_start(wt, wten[:, :, :])
                        for nh in range(2):
                            for j2 in range(CT // 256):
                                nc.tensor.matmul(
                                    kv_ps[:, ds(512 * nh, 512)],
                                    ehsT_sb[:, ds(2 * j2, 2), :],
                                    wt[:, ds(2 * j2, 2), ds(512 * nh, 512)],
                                    start=(j2 == 0),
                                    stop=(j2 == CT // 256 - 1),
                                    perf_mode=DR,
                                )
                        kvs = sa_sb.tile([T, C], BF16, tag=f"{name}sb", bufs=1)
                        # host scales Wk/Wv by 64 for fp8 range; fold out here
                        # (k also folds 1/QSC so scoresT = (k/64)^T (64 q/8))
                        sc = 1.0 / (QSC * 64.0) if name == "k" else 1.0 / 64.0
                        nc.scalar.activation(kvs, kv_ps[ds(0, T), :], AF.Copy, scale=sc)
                        kv_sb[name] = kvs

                    # kT / vT via PE transpose of 128-column slices
                    vT_sb = sa_sb.tile([128, C // 128, T], BF16, bufs=1)
                    for src, dst in ((kv_sb["k"], kT_sb), (kv_sb["v"], vT_sb)):
                        for i in range(C // 128):
                            tp = sa_ps.tile([128, T], BF16, tag="tpa")
                            nc.tensor.transpose(tp, src[:, ds(128 * i, 128)], ident[:T, :T])
                            nc.any.tensor_copy(dst[:, i, :], tp)

                    # M_h = v_h @ (16 Wo_h), fp8-packed at stacked row 77h.
                    # Head pairs run in disjoint row-strips on two psum banks.
                    wot_all = sa_w.tile([128, C // 128, C], BF16, tag="wot", bufs=1)
                    nc.sync.dma_start(wot_all, wo[:, :, :])
                    for i in range(C // 128):
                        wot = wot_all[:, i, :]
                        mps = [big_ps(), big_ps()]
                        for nh in range(2):
                            for sub in range(2):
                                nc.tensor.matmul(
                                    mps[sub][ds(0, T), ds(512 * nh, 512)],
                                    vT_sb[ds(64 * sub, 64), i, :],
                                    wot[ds(64 * sub, 64), ds(512 * nh, 512)],
                                    start=True,
                                    stop=True,
                                )
                        for sub in range(2):
                            h = 2 * i + sub
                            m_stg = sa_sb.tile([T, C], FP8, tag="mstg")
                            nc.any.tensor_copy(m_stg, mps[sub][ds(0, T), :])
                            for (ti, pb, s0, nr) in _pack_pieces(h):
                                nc.gpsimd.dma_start(
                                    m_f8[ds(pb, nr), ti, :], m_stg[ds(s0, nr), :]
                                )

                # ---------------- stage B ----------------
                with tc.tile_pool(name="ops", bufs=2, space="PSUM") as ops:

                    def chunk_eu(ci):
                        # shared suppression factor exp(col_bias)^T for this
                        # chunk: one K=2 matmul + one Exp, used by all heads
                        q0 = NQ * ci
                        b_ps = ops.tile([T, NQ], F32, tag="dexp", bufs=2, name="bps")
                        nc.tensor.matmul(
                            b_ps, ind2_sb, mask2_sb[:, ds(q0, NQ)], start=True, stop=True
                        )
                        euT = soft.tile([T, NQ], BF16, tag="euT", bufs=2)
                        nc.scalar.activation(euT, b_ps, AF.Exp)
                        return euT

                    def sm_pair(ci, pair, d_state, euT):
                        qT = st[ci]["qT"]
                        zb = z_bufs[ci % 2]
                        sps = []
                        for sub in range(2):
                            po = 64 * sub
                            sT_ps = ops.tile([T, NQ], F32, tag="sT", bufs=2)
                            nc.tensor.matmul(
                                sT_ps,
                                kT_sb[ds(po, 64), pair, :],
                                qT[ds(po, 64), pair, :],
                                start=True,
                                stop=True,
                            )
                            sps.append(sT_ps)
                        for sub in range(2):
                            h = 2 * pair + sub
                            z_h = soft.tile([T, NQ], FP8, tag="zh", bufs=4)
                            nc.scalar.activation(z_h, sps[sub], AF.Exp)
                            z2 = soft.tile([T, NQ], FP8, tag="z2", bufs=4)
                            nc.vector.tensor_mul(z2, z_h, euT)
                            for (ti, pb, s0, nr) in _pack_pieces(h):
                                nc.gpsimd.dma_start(
                                    zb[ds(pb, nr), ti, :], z2[ds(s0, nr), :]
                                )
                        # D matmuls (DoubleRow over k-tile pairs) as soon as
                        # both tiles of a pair are fully packed
                        zrows = 154 * (pair + 1)
                        while (
                            d_state["kt"] < NKT
                            and (128 * (d_state["kt"] + 2) <= zrows or pair == 7)
                        ):
                            kt = d_state["kt"]
                            nc.tensor.matmul(
                                d_state["ps"],
                                etd_sb[:, ds(kt, 2), :],
                                zb[:, ds(kt, 2), :],
                                start=(kt == 0),
                                stop=(kt == NKT - 2),
                                perf_mode=DR,
                            )
                            d_state["kt"] += 2

                    def emit_dinv(d_state):
                        dinv = soft.tile([16, NQ], F32, tag="dinv", bufs=2)
                        nc.vector.reciprocal_approx_fast(dinv, d_state["ps"][ds(0, 16), :])
                        dinv_bf = soft.tile([49, NQ], BF16, tag="dinvbf", bufs=2)
                        nc.any.memset(dinv_bf, 1.0)
                        nc.scalar.activation(
                            dinv_bf[ds(0, 16), :], dinv, AF.Copy, scale=PSC
                        )
                        nc.scalar.activation(
                            dinv_bf[ds(32, 16), :], dinv, AF.Copy, scale=PSC
                        )
                        return dinv_bf

                    def expand_norm(ci, dinv_bf):
                        zb = z_bufs[ci % 2]
                        pb = prob_bufs[ci % 2]
                        for kt in range(NKT):
                            # alternate row-strips 0/1 so consecutive expand
                            # matmuls overlap in the PE array
                            po = 32 * (kt % 2)
                            dexp_ps = ops.tile([128, NQ], F32, tag="dexp", bufs=2)
                            nc.tensor.matmul(
                                dexp_ps,
                                ex_sb[ds(po, 17), kt, :],
                                dinv_bf[ds(po, 17), :],
                                start=True,
                                stop=True,
                            )
                            nc.vector.tensor_mul(pb[:, kt, :], zb[:, kt, :], dexp_ps)

                    def av_group(ci, g):
                        qj, nh = g // 2, g % 2
                        pb = prob_bufs[ci % 2]
                        res_t = st[ci]["res"]
                        o_ps = ops.tile([128, 512], F32, tag="ops", bufs=2)
                        for p5 in range(NKT // 2):
                            nc.tensor.matmul(
                                o_ps,
                                pb[:, ds(2 * p5, 2), ds(128 * qj, 128)],
                                m_f8[:, ds(2 * p5, 2), ds(512 * nh, 512)],
                                start=(p5 == 0),
                                stop=(p5 == NKT // 2 - 1),
                                perf_mode=DR,
                            )
                        if g == 0:
                            st[ci]["osb"] = work.tile(
                                [128, NQ // 128, C], BF16, tag="osb", bufs=2,
                                name=f"osb{ci}",
                            )
                        o_sb = st[ci]["osb"]
                        nc.vector.scalar_tensor_tensor(
                            o_sb[:, qj, ds(512 * nh, 512)],
                            o_ps,
                            1.0 / (PSC * MSC),
                            res_t[:, qj, ds(512 * nh, 512)],
                            op0=ALU.mult,
                            op1=ALU.add,
                        )
                        if nh == 1:
                            nc.sync.dma_start(
                                out[:, ds(ci * (NQ // 128) + qj, 1), :],
                                o_sb[:, ds(qj, 1), :],
                            )

                    for ci in range(NCHUNK):
                        if ci + 1 < NCHUNK:
                            load(ci + 1)
                        d_state = {
                            "kt": 0,
                            "ps": ops.tile([32, NQ], F32, tag="dps", bufs=1, name="dps"),
                        }
                        euT = chunk_eu(ci)
                        for pair in range(H // 2):
                            if ci > 0:
                                av_group(ci - 1, pair)
                            sm_pair(ci, pair, d_state, euT)
                            if pair < 6 and ci + 1 < NCHUNK:
                                qt_group(ci + 1, pair, ops)
                        dinv_bf = emit_dinv(d_state)
                        if ci + 1 < NCHUNK:
                            qt_group(ci + 1, 6, ops)
                            qt_group(ci + 1, 7, ops)
                        expand_norm(ci, dinv_bf)
                    for g in range(8):
                        av_group(NCHUNK - 1, g)

    nc.compile()
    return nc


_NC_CACHE = {}


def get_nc():
    if "nc" not in _NC_CACHE:
        _NC_CACHE["nc"] = build_nc()
    return _NC_CACHE["nc"]


def _bf16(x):
    return np.asarray(x, dtype=ml_dtypes.bfloat16)


def _fp8(x):
    return np.clip(np.asarray(x, np.float32), -240.0, 240.0).astype(
        ml_dtypes.float8_e4m3
    )


def make_in_maps(inputs):
    hs = np.asarray(inputs["hidden_states"], dtype=np.float32)
    ehs = np.asarray(inputs["encoder_hidden_states"], dtype=np.float32)
    mask_A = np.asarray(inputs["mask_A"], dtype=np.float32)
    mask_B = np.asarray(inputs["mask_B"], dtype=np.float32)
    Wq = np.asarray(inputs["Wq"], dtype=np.float32)
    Wk = np.asarray(inputs["Wk"], dtype=np.float32)
    Wv = np.asarray(inputs["Wv"], dtype=np.float32)
    Wo = np.asarray(inputs["Wo"], dtype=np.float32)
    bo = np.asarray(inputs["bo"], dtype=np.float32)
    idxA = np.asarray(inputs["token_indices_A"]).astype(np.int64) % T
    idxB = np.asarray(inputs["token_indices_B"]).astype(np.int64) % T

    # rank-2 suppression: bias[t,q] = ind2[:,t] . mask2[:,q], with B-set
    # overwriting A-set (reference applies A then B)
    inA = np.zeros(T, np.float32)
    inA[idxA] = 1.0
    inB = np.zeros(T, np.float32)
    inB[idxB] = 1.0
    ind2_np = np.stack([-SUPPRESS * inA * (1.0 - inB), -SUPPRESS * inB])
    mask2_np = np.stack([1.0 - mask_A, 1.0 - mask_B])

    # D-sum indicator [p, kt*16+h] and expand indicator [h(17), kt*128+p]
    rows = np.arange(NKT * 128)
    head_of = np.where(rows < H * T, rows // T, -1)
    etd_np = np.zeros((128, NKT * 32), np.float32)
    ex_np = np.zeros((17, NKT * 128), np.float32)
    for kt in range(NKT):
        for p in range(128):
            hh = head_of[kt * 128 + p]
            if 0 <= hh < H:
                etd_np[p, kt * 32 + hh] = 1.0
                ex_np[hh, kt * 128 + p] = 1.0
    ex_np[16, BO_TILE * 128 + BO_PART] = 1.0

    def _tile3(a):
        # [R, C] -> [128, R//128, C] with row r = 128*j + p at [p, j, :]
        R = a.shape[0]
        return np.ascontiguousarray(
            a.reshape(R // 128, 128, a.shape[1]).transpose(1, 0, 2)
        )

    wq_f8 = _fp8(_tile3(Wq * (QSC / np.sqrt(D))))
    wk_f8, wv_f8 = _fp8(_tile3(Wk * 64.0)), _fp8(_tile3(Wv * 64.0))
    wo_bf = _bf16(_tile3(Wo * MSC))
    bo_f8 = _fp8(bo * MSC)[None, :]
    ind2_bf, mask2_bf = _bf16(ind2_np), _bf16(mask2_np)
    etd_bf, ex_bf = _fp8(etd_np), _bf16(ex_np)

    ehsT_f8 = np.zeros((B, 128, CT // 128, 80), ml_dtypes.float8_e4m3)
    for b in range(B):
        ehsT_f8[b, :, :, :T] = _fp8(_tile3(ehs[b].T.copy()))

    in_maps = []
    for b in range(B):
        in_maps.append(
            {
                "hsT": _fp8(_tile3(hs[b].T)),
                "hsres": _bf16(_tile3(hs[b])),
                "ehsT": ehsT_f8[b],
                "wq": wq_f8,
                "wk": wk_f8,
                "wv": wv_f8,
                "wo": wo_bf,
                "bo": bo_f8,
                "ind2": ind2_bf,
                "mask2": mask2_bf,
                "etd": etd_bf,
                "exp_ind": ex_bf,
            }
        )
    return in_maps


def kernel(**inputs) -> np.ndarray:
    from concourse.bass_utils import run_bass_kernel_spmd

    nc = get_nc()
    in_maps = make_in_maps(inputs)
    res = run_bass_kernel_spmd(nc, in_maps, core_ids=list(range(B)))
    outs = []
    for b in range(B):
        o = np.asarray(res.results[b]["out"])  # [128, 32, 1024]
        outs.append(o.transpose(1, 0, 2).reshape(HW, C))
    return np.stack(outs).astype(np.float32)


# revision 24
# speedup vs baseline: 1.2348x; 1.0133x over previous
"""Trainium2 Bass kernel for nn_DenseAttnProcessor (sparse_attention).

Cross-attention block: q = hs@Wq, k/v = ehs@{Wk,Wv}, per-head softmax((q k^T)/8
+ col_bias) @ v, @Wo + bo + residual.  B=8 batches -> data-parallel, one batch
per NeuronCore (no collectives).  716us baseline -> 418us.

Per-core dataflow:

  host prep:  all operands pre-tiled to [128, ntiles, free] so every load is
              ONE DMA (the ~680ns per-dma issue cost on the Sync sequencer was
              a bottleneck); hs pre-transposed to fp8 hsT (no runtime
              DMA-transpose); residual bf16; Wq*8 fp8, Wk/Wv*64 fp8, Wo*16
              bf16; suppression bias factored rank-2 (ind2 [2,77] x mask2
              [2,HW], exact set-overwrite semantics); head-indicator matrices
              for the batched softmax denominator.
  stage A:    k,v = ehsT^T @ {Wk,Wv} (fp8 DoubleRow, 2 k-tiles/matmul);
              kT = PE-transpose(k)/4096; M_h = v_h @ (16 Wo_h) (head pairs in
              disjoint array row-strips overlap); M packed fp8 into
              [128,10,1024] stacked tiles (+16*bo row at stacked row 1232).
  stage B (8 chunks of 512 q rows, software-pipelined; AV of chunk ci runs
  inside the head loop of chunk ci+1 so the PE stream stays dense):
    euT  = Exp(ind2^T @ mask2-chunk)  -- one K=2 matmul + Exp per chunk
    qT   = (8Wq)^T @ hsT              -- fp8 DoubleRow
    per head pair: scoresT [77,512] = kT_h^T qT_h (K=64; the pair runs in
         disjoint row-strips / psum banks and overlaps)
    z    = Exp(scoresT) fp8 * euT (DVE) -> packed into stacked [128,10,512]
         (gpsimd dma), D matmuls chase the packs incrementally
    D    = indicator^T @ zstack       -- 5 fp8-DoubleRow matmuls, [16,512]
    dinv = 32/D (fast DVE reciprocal)
    dexp = expand-indicator^T @ dinv per k-tile (alternating row-strips)
    prob = z * dexp  (DVE, fp8, = 32*softmax)
    AV   = prob^T @ M                 -- fp8 DoubleRow, 5 matmuls per psum
    out  = psum/512 + residual (fused DVE scalar_tensor_tensor), one bf16
           store per chunk in pre-tiled layout (host untiles).
"""

import sys

for _p in ("/opt/trn_rl_repo",):
    if _p not in sys.path:
        sys.path.insert(0, _p)

import numpy as np
import ml_dtypes

import concourse.mybir as mybir
import concourse.tile as tile
from concourse import bacc
from concourse.bass import ds
from concourse.masks import make_identity

F32 = mybir.dt.float32
BF16 = mybir.dt.bfloat16
FP8 = mybir.dt.float8e4
AF = mybir.ActivationFunctionType
ALU = mybir.AluOpType
DR = mybir.MatmulPerfMode.DoubleRow

B, HW, C, CT, T, H, D = 8, 4096, 1024, 2048, 77, 16, 64
SUPPRESS = 20.0
RT = H * T + 1                # 1233 stacked rows (16*77 head rows + bo row)
NKT = (RT + 127) // 128       # 10 K-tiles for the AV matmul
NQ = 512                      # q rows per chunk
NCHUNK = HW // NQ             # 8
BO_TILE, BO_PART = (H * T) // 128, (H * T) % 128   # bo/ones row: tile 9, p 80
PSC = 32.0                    # probs scale (fp8 range)
MSC = 16.0                    # M scale (fp8 range)
QSC = 64.0                    # q scale (Wq*scale*64 fp8, kT/64)


def _pack_pieces(h):
    """DMA pieces for packing head h's 77 rows at stacked row 77*h, split at
    128-row tile boundaries.  Returns list of (tile_idx, part_base, src_start,
    nrows)."""
    g = T * h
    pieces = []
    pos = 0
    while pos < T:
        gg = g + pos
        ti, d = gg // 128, gg % 128
        n = min(T - pos, 128 - d)
        pieces.append((ti, d, pos, n))
        pos += n
    return pieces


def build_nc():
    nc = bacc.Bacc("TRN2", target_bir_lowering=False, debug=False)

    hsT = nc.dram_tensor("hsT", [128, C // 128, HW], FP8, kind="ExternalInput")
    hsres = nc.dram_tensor("hsres", [128, HW // 128, C], BF16, kind="ExternalInput")
    ehsT = nc.dram_tensor("ehsT", [128, CT // 128, 80], FP8, kind="ExternalInput")
    wq = nc.dram_tensor("wq", [128, C // 128, C], FP8, kind="ExternalInput")
    wk = nc.dram_tensor("wk", [128, CT // 128, C], FP8, kind="ExternalInput")
    wv = nc.dram_tensor("wv", [128, CT // 128, C], FP8, kind="ExternalInput")
    wo = nc.dram_tensor("wo", [128, C // 128, C], BF16, kind="ExternalInput")
    bo = nc.dram_tensor("bo", [1, C], FP8, kind="ExternalInput")
    ind2 = nc.dram_tensor("ind2", [2, T], BF16, kind="ExternalInput")
    mask2 = nc.dram_tensor("mask2", [2, HW], BF16, kind="ExternalInput")
    etd = nc.dram_tensor("etd", [128, NKT * 32], FP8, kind="ExternalInput")
    exp_ind = nc.dram_tensor("exp_ind", [17, NKT * 128], BF16, kind="ExternalInput")
    out = nc.dram_tensor("out", [128, HW // 128, C], BF16, kind="ExternalOutput")

    with tile.TileContext(nc) as tc:
        with (
            tc.tile_pool(name="const", bufs=1) as const,
            tc.tile_pool(name="persist", bufs=1) as persist,
        ):
            ident = const.tile([128, 128], BF16)
            make_identity(nc, ident)
            ind2_sb = const.tile([2, T], BF16)
            nc.sync.dma_start(ind2_sb, ind2[:, :])
            mask2_sb = const.tile([2, HW], BF16)
            nc.sync.dma_start(mask2_sb, mask2[:, :])
            etd_sb = const.tile([128, NKT, 32], FP8)
            nc.sync.dma_start(etd_sb, etd[:, :])
            ex_sb = const.tile([49, NKT, 128], BF16)
            nc.sync.dma_start(ex_sb[ds(0, 17), :, :], exp_ind[:, :])
            nc.sync.dma_start(ex_sb[ds(32, 17), :, :], exp_ind[:, :])

            # persistent stacks
            kT_sb = persist.tile([128, C // 128, T], BF16)        # [inner, t]
            m_f8 = persist.tile([128, NKT, C], FP8)               # stacked 16*M
            wq_sb = persist.tile([128, C // 128, C], FP8)
            nc.sync.dma_start(wq_sb, wq[:, :, :])
            z_bufs = [persist.tile([128, NKT, NQ], FP8, name=f"z{b}") for b in range(2)]
            prob_bufs = [persist.tile([128, NKT, NQ], FP8, name=f"pb{b}") for b in range(2)]
            psc_row = const.tile([1, NQ], FP8)
            nc.any.memset(psc_row, PSC)
            for zb in z_bufs:
                # bo/ones pseudo-row = PSC; rows past it zero (NaN hygiene for
                # the D matmul which reads all 128 partitions).  memset can
                # only start at 32-aligned partitions; DMA patches row 80.
                nc.any.memset(zb[ds(64, 64), BO_TILE, :], 0.0)
                nc.sync.dma_start(zb[ds(BO_PART, 1), BO_TILE, :], psc_row)
            # M stack tile 9: rows past head rows; bo*16 at BO_PART
            nc.any.memset(m_f8[ds(64, 64), BO_TILE, :], 0.0)
            nc.sync.dma_start(m_f8[ds(BO_PART, 1), BO_TILE, :], bo[:, :])

            st = {}

            with (
                tc.tile_pool(name="hsp", bufs=2) as hsp,
                tc.tile_pool(name="work", bufs=2) as work,
                tc.tile_pool(name="soft", bufs=4) as soft,
            ):

                def load(ci):
                    q0 = NQ * ci
                    hsT_t = hsp.tile([128, C // 128, NQ], FP8, tag="hsT")
                    for hh in range(2):
                        nc.sync.dma_start(
                            hsT_t[:, ds(4 * hh, 4), :],
                            hsT[:, ds(4 * hh, 4), ds(q0, NQ)],
                        )
                    res_t = hsp.tile([128, NQ // 128, C], BF16, tag="res", bufs=3)
                    for hh in range(2):
                        nc.sync.dma_start(
                            res_t[:, ds(2 * hh, 2), :],
                            hsres[:, ds(ci * (NQ // 128) + 2 * hh, 2), :],
                        )
                    qT = work.tile([128, C // 128, NQ], BF16, tag="qT")
                    st[ci] = dict(hsT=hsT_t, res=res_t, qT=qT)

                def qt_group(ci, ij, ps_pool, tag="qps", bufs=1):
                    hsT_t, qT = st[ci]["hsT"], st[ci]["qT"]
                    q_ps = ps_pool.tile([128, NQ], F32, tag=tag, bufs=bufs)
                    for c2 in range(C // 256):
                        nc.tensor.matmul(
                            q_ps,
                            wq_sb[:, ds(2 * c2, 2), ds(128 * ij, 128)],
                            hsT_t[:, ds(2 * c2, 2), :],
                            start=(c2 == 0),
                            stop=(c2 == C // 256 - 1),
                            perf_mode=DR,
                        )
                    nc.any.tensor_copy(qT[:, ij, :], q_ps)

                # ---------------- stage A: k, v, kT, M ----------------
                with (
                    tc.tile_pool(name="sa_sb", bufs=3) as sa_sb,
                    tc.tile_pool(name="sa_w", bufs=4) as sa_w,
                    tc.tile_pool(name="sa_ps", bufs=2, space="PSUM") as sa_ps,
                ):
                    # chunk-0 loads + qT(0) early so PE/DMA warm up while the
                    # k/v weight tiles stream in.  "big" [128,1024] psum tag is
                    # shared by qT(0) (first 512 cols) and the M matmuls.
                    load(0)
                    ehsT_sb = sa_sb.tile([128, CT // 128, 80], FP8, bufs=1)
                    nc.sync.dma_start(ehsT_sb, ehsT[:, :, :])

                    def big_ps():
                        return sa_ps.tile([128, C], F32, tag="big", bufs=2, name="bigps")

                    for ij in range(C // 128):
                        hsT_t, qT = st[0]["hsT"], st[0]["qT"]
                        q_ps = big_ps()
                        for c2 in range(C // 256):
                            nc.tensor.matmul(
                                q_ps[:, ds(0, NQ)],
                                wq_sb[:, ds(2 * c2, 2), ds(128 * ij, 128)],
                                hsT_t[:, ds(2 * c2, 2), :],
                                start=(c2 == 0),
                                stop=(c2 == C // 256 - 1),
                                perf_mode=DR,
                            )
                        nc.any.tensor_copy(qT[:, ij, :], q_ps[:, ds(0, NQ)])

                    kv_sb = {}
                    for name, wten in (("k", wk), ("v", wv)):
                        kv_ps = sa_ps.tile([80, C], F32, tag="kvps", bufs=1)
                        wt = sa_w.tile([128, CT // 128, C], FP8, tag=f"w{name}", bufs=1)
                        nc.sync.dma_start(wt, wten[:, :, :])
                        for nh in range(2):
                            for j2 in range(CT // 256):
                                nc.tensor.matmul(
                                    kv_ps[:, ds(512 * nh, 512)],
                                    ehsT_sb[:, ds(2 * j2, 2), :],
                                    wt[:, ds(2 * j2, 2), ds(512 * nh, 512)],
                                    start=(j2 == 0),
                                    stop=(j2 == CT // 256 - 1),
                                    perf_mode=DR,
                                )
                        kvs = sa_sb.tile([T, C], BF16, tag=f"{name}sb", bufs=1)
                        # host scales Wk/Wv by 64 for fp8 range; fold out here
                        # (k also folds 1/QSC so scoresT = (k/64)^T (64 q/8))
                        sc = 1.0 / (QSC * 64.0) if name == "k" else 1.0 / 64.0
                        nc.scalar.activation(kvs, kv_ps[ds(0, T), :], AF.Copy, scale=sc)
                        kv_sb[name] = kvs

                    # kT / vT via PE transpose of 128-column slices
                    vT_sb = sa_sb.tile([128, C // 128, T], BF16, bufs=1)
                    for src, dst in ((kv_sb["k"], kT_sb), (kv_sb["v"], vT_sb)):
                        for i in range(C // 128):
                            tp = sa_ps.tile([128, T], BF16, tag="tpa")
                            nc.tensor.transpose(tp, src[:, ds(128 * i, 128)], ident[:T, :T])
                            nc.any.tensor_copy(dst[:, i, :], tp)

                    # M_h = v_h @ (16 Wo_h), fp8-packed at stacked row 77h.
                    # Head pairs run in disjoint row-strips on two psum banks.
                    wot_all = sa_w.tile([128, C // 128, C], BF16, tag="wot", bufs=1)
                    nc.sync.dma_start(wot_all, wo[:, :, :])
                    for i in range(C // 128):
                        wot = wot_all[:, i, :]
                        mps = [big_ps(), big_ps()]
                        for nh in range(2):
                            for sub in range(2):
                                nc.tensor.matmul(
                                    mps[sub][ds(0, T), ds(512 * nh, 512)],
                                    vT_sb[ds(64 * sub, 64), i, :],
                                    wot[ds(64 * sub, 64), ds(512 * nh, 512)],
                                    start=True,
                                    stop=True,
                                )
                        for sub in range(2):
                            h = 2 * i + sub
                            m_stg = sa_sb.tile([T, C], FP8, tag="mstg")
                            nc.any.tensor_copy(m_stg, mps[sub][ds(0, T), :])
                            for (ti, pb, s0, nr) in _pack_pieces(h):
                                nc.gpsimd.dma_start(
                                    m_f8[ds(pb, nr), ti, :], m_stg[ds(s0, nr), :]
                                )

                # ---------------- stage B ----------------
                with tc.tile_pool(name="ops", bufs=2, space="PSUM") as ops:

                    def chunk_eu(ci):
                        # shared suppression factor exp(col_bias)^T for this
                        # chunk: one K=2 matmul + one Exp, used by all heads
                        q0 = NQ * ci
                        b_ps = ops.tile([T, NQ], F32, tag="dexp", bufs=2, name="bps")
                        nc.tensor.matmul(
                            b_ps, ind2_sb, mask2_sb[:, ds(q0, NQ)], start=True, stop=True
                        )
                        euT = soft.tile([T, NQ], BF16, tag="euT", bufs=2)
                        nc.scalar.activation(euT, b_ps, AF.Exp)
                        return euT

                    def sm_pair(ci, pair, d_state, euT):
                        qT = st[ci]["qT"]
                        zb = z_bufs[ci % 2]
                        sps = []
                        for sub in range(2):
                            po = 64 * sub
                            sT_ps = ops.tile([T, NQ], F32, tag="sT", bufs=2)
                            nc.tensor.matmul(
                                sT_ps,
                                kT_sb[ds(po, 64), pair, :],
                                qT[ds(po, 64), pair, :],
                                start=True,
                                stop=True,
                            )
                            sps.append(sT_ps)
                        for sub in range(2):
                            h = 2 * pair + sub
                            z_h = soft.tile([T, NQ], FP8, tag="zh", bufs=4)
                            nc.scalar.activation(z_h, sps[sub], AF.Exp)
                            z2 = soft.tile([T, NQ], FP8, tag="z2", bufs=4)
                            nc.vector.tensor_mul(z2, z_h, euT)
                            for (ti, pb, s0, nr) in _pack_pieces(h):
                                nc.gpsimd.dma_start(
                                    zb[ds(pb, nr), ti, :], z2[ds(s0, nr), :]
                                )
                        # D matmuls (DoubleRow over k-tile pairs), one pair
                        # late so the exp->mul->pack chain never stalls the
                        # PE queue; the rest flushes after the qt groups
                        zrows = 154 * pair
                        while (
                            d_state["kt"] < NKT
                            and 128 * (d_state["kt"] + 2) <= zrows
                        ):
                            kt = d_state["kt"]
                            nc.tensor.matmul(
                                d_state["ps"],
                                etd_sb[:, ds(kt, 2), :],
                                zb[:, ds(kt, 2), :],
                                start=(kt == 0),
                                stop=(kt == NKT - 2),
                                perf_mode=DR,
                            )
                            d_state["kt"] += 2

                    def emit_dinv(d_state):
                        dinv = soft.tile([16, NQ], F32, tag="dinv", bufs=2)
                        nc.vector.reciprocal_approx_fast(dinv, d_state["ps"][ds(0, 16), :])
                        dinv_bf = soft.tile([49, NQ], BF16, tag="dinvbf", bufs=2)
                        nc.any.memset(dinv_bf, 1.0)
                        nc.scalar.activation(
                            dinv_bf[ds(0, 16), :], dinv, AF.Copy, scale=PSC
                        )
                        nc.scalar.activation(
                            dinv_bf[ds(32, 16), :], dinv, AF.Copy, scale=PSC
                        )
                        return dinv_bf

                    def expand_norm(ci, dinv_bf):
                        zb = z_bufs[ci % 2]
                        pb = prob_bufs[ci % 2]
                        for kt in range(NKT):
                            # alternate row-strips 0/1 so consecutive expand
                            # matmuls overlap in the PE array
                            po = 32 * (kt % 2)
                            dexp_ps = ops.tile([128, NQ], F32, tag="dexp", bufs=2)
                            nc.tensor.matmul(
                                dexp_ps,
                                ex_sb[ds(po, 17), kt, :],
                                dinv_bf[ds(po, 17), :],
                                start=True,
                                stop=True,
                            )
                            nc.vector.tensor_mul(pb[:, kt, :], zb[:, kt, :], dexp_ps)

                    def av_group(ci, g):
                        qj, nh = g // 2, g % 2
                        pb = prob_bufs[ci % 2]
                        res_t = st[ci]["res"]
                        o_ps = ops.tile([128, 512], F32, tag="ops", bufs=2)
                        for p5 in range(NKT // 2):
                            nc.tensor.matmul(
                                o_ps,
                                pb[:, ds(2 * p5, 2), ds(128 * qj, 128)],
                                m_f8[:, ds(2 * p5, 2), ds(512 * nh, 512)],
                                start=(p5 == 0),
                                stop=(p5 == NKT // 2 - 1),
                                perf_mode=DR,
                            )
                        if g == 0:
                            st[ci]["osb"] = work.tile(
                                [128, NQ // 128, C], BF16, tag="osb", bufs=2,
                                name=f"osb{ci}",
                            )
                        o_sb = st[ci]["osb"]
                        nc.vector.scalar_tensor_tensor(
                            o_sb[:, qj, ds(512 * nh, 512)],
                            o_ps,
                            1.0 / (PSC * MSC),
                            res_t[:, qj, ds(512 * nh, 512)],
                            op0=ALU.mult,
                            op1=ALU.add,
                        )
                        if nh == 1:
                            nc.sync.dma_start(
                                out[:, ds(ci * (NQ // 128) + qj, 1), :],
                                o_sb[:, ds(qj, 1), :],
                            )

                    for ci in range(NCHUNK):
                        if ci + 1 < NCHUNK:
                            load(ci + 1)
                        d_state = {
                            "kt": 0,
                            "ps": ops.tile([32, NQ], F32, tag="dps", bufs=1, name="dps"),
                        }
                        euT = chunk_eu(ci)
                        for pair in range(H // 2):
                            if ci > 0:
                                av_group(ci - 1, pair)
                            sm_pair(ci, pair, d_state, euT)
                            if pair < 6 and ci + 1 < NCHUNK:
                                qt_group(ci + 1, pair, ops)
                        if ci + 1 < NCHUNK:
                            qt_group(ci + 1, 6, ops)
                            qt_group(ci + 1, 7, ops)
                        zb = z_bufs[ci % 2]
                        while d_state["kt"] < NKT:
                            kt = d_state["kt"]
                            nc.tensor.matmul(
                                d_state["ps"],
                                etd_sb[:, ds(kt, 2), :],
                                zb[:, ds(kt, 2), :],
                                start=(kt == 0),
                                stop=(kt == NKT - 2),
                                perf_mode=DR,
                            )
                            d_state["kt"] += 2
                        dinv_bf = emit_dinv(d_state)
                        expand_norm(ci, dinv_bf)
                    for g in range(8):
                        av_group(NCHUNK - 1, g)

    nc.compile()
    return nc


_NC_CACHE = {}


def get_nc():
    if "nc" not in _NC_CACHE:
        _NC_CACHE["nc"] = build_nc()
    return _NC_CACHE["nc"]


def _bf16(x):
    return np.asarray(x, dtype=ml_dtypes.bfloat16)


def _fp8(x):
    return np.clip(np.asarray(x, np.float32), -240.0, 240.0).astype(
        ml_dtypes.float8_e4m3
    )


def make_in_maps(inputs):
    hs = np.asarray(inputs["hidden_states"], dtype=np.float32)
    ehs = np.asarray(inputs["encoder_hidden_states"], dtype=np.float32)
    mask_A = np.asarray(inputs["mask_A"], dtype=np.float32)
    mask_B = np.asarray(inputs["mask_B"], dtype=np.float32)
    Wq = np.asarray(inputs["Wq"], dtype=np.float32)
    Wk = np.asarray(inputs["Wk"], dtype=np.float32)
    Wv = np.asarray(inputs["Wv"], dtype=np.float32)
    Wo = np.asarray(inputs["Wo"], dtype=np.float32)
    bo = np.asarray(inputs["bo"], dtype=np.float32)
    idxA = np.asarray(inputs["token_indices_A"]).astype(np.int64) % T
    idxB = np.asarray(inputs["token_indices_B"]).astype(np.int64) % T

    # rank-2 suppression: bias[t,q] = ind2[:,t] . mask2[:,q], with B-set
    # overwriting A-set (reference applies A then B)
    inA = np.zeros(T, np.float32)
    inA[idxA] = 1.0
    inB = np.zeros(T, np.float32)
    inB[idxB] = 1.0
    ind2_np = np.stack([-SUPPRESS * inA * (1.0 - inB), -SUPPRESS * inB])
    mask2_np = np.stack([1.0 - mask_A, 1.0 - mask_B])

    # D-sum indicator [p, kt*16+h] and expand indicator [h(17), kt*128+p]
    rows = np.arange(NKT * 128)
    head_of = np.where(rows < H * T, rows // T, -1)
    etd_np = np.zeros((128, NKT * 32), np.float32)
    ex_np = np.zeros((17, NKT * 128), np.float32)
    for kt in range(NKT):
        for p in range(128):
            hh = head_of[kt * 128 + p]
            if 0 <= hh < H:
                etd_np[p, kt * 32 + hh] = 1.0
                ex_np[hh, kt * 128 + p] = 1.0
    ex_np[16, BO_TILE * 128 + BO_PART] = 1.0

    def _tile3(a):
        # [R, C] -> [128, R//128, C] with row r = 128*j + p at [p, j, :]
        R = a.shape[0]
        return np.ascontiguousarray(
            a.reshape(R // 128, 128, a.shape[1]).transpose(1, 0, 2)
        )

    wq_f8 = _fp8(_tile3(Wq * (QSC / np.sqrt(D))))
    wk_f8, wv_f8 = _fp8(_tile3(Wk * 64.0)), _fp8(_tile3(Wv * 64.0))
    wo_bf = _bf16(_tile3(Wo * MSC))
    bo_f8 = _fp8(bo * MSC)[None, :]
    ind2_bf, mask2_bf = _bf16(ind2_np), _bf16(mask2_np)
    etd_bf, ex_bf = _fp8(etd_np), _bf16(ex_np)

    ehsT_f8 = np.zeros((B, 128, CT // 128, 80), ml_dtypes.float8_e4m3)
    for b in range(B):
        ehsT_f8[b, :, :, :T] = _fp8(_tile3(ehs[b].T.copy()))

    in_maps = []
    for b in range(B):
        in_maps.append(
            {
                "hsT": _fp8(_tile3(hs[b].T)),
                "hsres": _bf16(_tile3(hs[b])),
                "ehsT": ehsT_f8[b],
                "wq": wq_f8,
                "wk": wk_f8,
                "wv": wv_f8,
                "wo": wo_bf,
                "bo": bo_f8,
                "ind2": ind2_bf,
                "mask2": mask2_bf,
                "etd": etd_bf,
                "exp_ind": ex_bf,
            }
        )
    return in_maps


def kernel(**inputs) -> np.ndarray:
    from concourse.bass_utils import run_bass_kernel_spmd

    nc = get_nc()
    in_maps = make_in_maps(inputs)
    res = run_bass_kernel_spmd(nc, in_maps, core_ids=list(range(B)))
    outs = []
    for b in range(B):
        o = np.asarray(res.results[b]["out"])  # [128, 32, 1024]
        outs.append(o.transpose(1, 0, 2).reshape(HW, C))
    return np.stack(outs).astype(np.float32)
